# revision 1
# baseline (speedup 1.0000x reference)
"""GCN layer (message passing) on 8 Trainium2 NeuronCores.

out = relu(((D^-1/2 A D^-1/2) X) @ W.T) + X

Strategy (dst-sharded graph partitioning):
  - Destination nodes sharded across 8 cores (12500 nodes each); every core
    holds the full feature table (random-access gather source) and computes
    its 12500 output rows; the host concatenates.
  - Host-side prep (index-space only): per-edge weight ns2 = norm[src]*norm[dst]
    (both rsqrt-degree norms folded into the edge weight); edges grouped by
    (dst tile of 128 nodes, src bucket of 25000 nodes, src) so each dst tile's
    sources are gathered with dma_gather (int16 indices => src buckets), with
    ascending addresses per stream for HBM locality.
  - Device, per dst tile: up to 4 dma_gather calls pull all edge source rows
    into X (the dominant memory traffic ~216MB/core). The segment-sum runs on
    the tensor engine as  zT[i,d] += X_c[e,i].T @ S_c[e,d]  where
    S_c[e,d] = (d == local_dst[e]) * ns2[e] is built with one fused
    tensor_scalar (is_equal then mult) against a constant iota row matrix.
    Then y[d,o] = zT.T @ W.T on the PE, ReLU on ACT, residual add on DVE.
  - num_idxs per gather is static per (tile, bucket) = max count over the 8
    cores (SPMD same-program constraint), so padding is only the cross-core
    spread (~5%); pad slots gather row 0 of the bucket and are annihilated by
    local_dst = -1 (one-hot row of zeros). Unwritten tail columns of X are
    killed the same way, but the first X pool slots are memzeroed once since
    0 * garbage-NaN would poison PSUM.
"""

import math

import numpy as np

import concourse.bacc as bacc
import concourse.mybir as mybir
from concourse.bass_utils import run_bass_kernel_spmd
from concourse.tile import TileContext

P = 128
N_CORES = 8
BUCKET_MAX = 25000  # int16 gather indices: bucket the node space


def _prepare(features, W, edge_src, edge_dst, n_cores=N_CORES, bucket_max=BUCKET_MAX):
    """Partition the graph by dst core / dst tile / src bucket."""
    features = np.asarray(features, dtype=np.float32)
    W = np.asarray(W, dtype=np.float32)
    edge_src = np.asarray(edge_src, dtype=np.int32)
    edge_dst = np.asarray(edge_dst, dtype=np.int32)

    n_nodes, d = features.shape
    assert d == P
    assert n_nodes % n_cores == 0
    npc = n_nodes // n_cores
    n_tiles = math.ceil(npc / P)
    rows_last = npc - (n_tiles - 1) * P
    nb = math.ceil(n_nodes / bucket_max)
    B = math.ceil(n_nodes / nb)
    assert B <= 32768

    degs = np.bincount(edge_dst, minlength=n_nodes).astype(np.float32)
    norm = 1.0 / np.sqrt(np.maximum(degs, 1.0), dtype=np.float32)
    ns2 = norm[edge_src] * norm[edge_dst]

    core_of = edge_dst // npc

    # first pass: per-core sorted edge lists and per-(tile,bucket) counts
    per_core = []
    counts_all = np.zeros((n_cores, n_tiles, nb), np.int64)
    for k in range(n_cores):
        sel = np.flatnonzero(core_of == k)
        src_k = edge_src[sel]
        ldst = edge_dst[sel] - k * npc
        tile_of = ldst // P
        bucket = src_k // B
        order = np.lexsort((src_k, bucket, tile_of))
        sel = sel[order]
        gid = tile_of[order] * nb + bucket[order]
        counts = np.bincount(gid, minlength=n_tiles * nb).reshape(n_tiles, nb)
        counts_all[k] = counts
        per_core.append((sel, gid, (ldst[order] % P).astype(np.float32)))

    # static per-(tile,bucket) gather sizes: max across cores
    n_tb = counts_all.max(axis=0)  # [n_tiles, nb]
    ct_tb = (n_tb + P - 1) // P  # chunks per (tile, bucket)
    C_t = ct_tb.sum(axis=1)  # chunks per tile
    icols_tb = (n_tb + 15) // 16  # int16 idx columns per (tile, bucket)
    icols_t = icols_tb.sum(axis=1)

    # column offsets in the packed DRAM arrays
    chunk_off_in_tile = np.cumsum(ct_tb, axis=1) - ct_tb  # [n_tiles, nb]
    icol_off_in_tile = np.cumsum(icols_tb, axis=1) - icols_tb
    ldns_col_off = np.concatenate([[0], np.cumsum(3 * C_t)])[:-1]  # per tile
    icol_off_tile = np.concatenate([[0], np.cumsum(icols_t)])[:-1]
    total_icols = int(icols_t.sum())
    total_ldns = int((3 * C_t).sum())

    layout = dict(
        n_nodes=n_nodes,
        npc=npc,
        n_tiles=n_tiles,
        rows_last=rows_last,
        nb=nb,
        B=B,
        n_tb=n_tb,
        ct_tb=ct_tb,
        C_t=C_t,
        icols_tb=icols_tb,
        chunk_off_in_tile=chunk_off_in_tile,
        icol_off_in_tile=icol_off_in_tile,
        ldns_col_off=ldns_col_off,
        icol_off_tile=icol_off_tile,
        total_icols=total_icols,
        total_ldns=total_ldns,
    )

    in_maps = []
    wt = np.ascontiguousarray(W.T)  # wt[i, o] = W[o, i]
    iotam = np.tile(np.arange(P, dtype=np.float32), (P, 1))
    for k in range(n_cores):
        sel, gid, ld_sorted = per_core[k]
        group_start = np.zeros(n_tiles * nb, np.int64)
        cnts = counts_all[k].reshape(-1)
        group_start[1:] = np.cumsum(cnts)[:-1]
        pos = np.arange(len(sel)) - group_start[gid]
        t_of = gid // nb
        b_of = gid % nb

        # idx array [16, total_icols] then replicated to 128 partitions
        idx16 = np.zeros((16, total_icols), np.int16)
        icol = icol_off_tile[t_of] + icol_off_in_tile[t_of, b_of] + pos // 16
        idx16[pos % 16, icol] = (edge_src[sel] - b_of * B).astype(np.int16)
        idxm = np.tile(idx16, (8, 1))

        # ldns array [128, total_ldns]: per tile [ld columns | ns columns]
        ldns = np.zeros((P, total_ldns), np.float32)
        # default ld = -1 in all ld column regions
        for t in range(n_tiles):
            ldns[:, ldns_col_off[t] : ldns_col_off[t] + C_t[t]] = -1.0
        cit = chunk_off_in_tile[t_of, b_of] + pos // P
        e_idx = pos % P
        ldns[e_idx, ldns_col_off[t_of] + cit] = ld_sorted
        ldns[e_idx, ldns_col_off[t_of] + C_t[t_of] + cit] = ns2[sel]
        ldns[e_idx, ldns_col_off[t_of] + 2 * C_t[t_of] + cit] = -ns2[sel]

        in_maps.append(
            {
                "feats": features,
                "idxm": np.ascontiguousarray(idxm),
                "ldns": np.ascontiguousarray(ldns),
                "wt": wt,
                "iotam": iotam,
                "resid": np.ascontiguousarray(features[k * npc : (k + 1) * npc]),
            }
        )
    return in_maps, layout


def _build_program(layout):
    f32 = mybir.dt.float32
    i16 = mybir.dt.int16
    n_nodes = layout["n_nodes"]
    npc = layout["npc"]
    n_tiles = layout["n_tiles"]
    rows_last = layout["rows_last"]
    nb = layout["nb"]
    B = layout["B"]
    n_tb = layout["n_tb"]
    ct_tb = layout["ct_tb"]
    C_t = layout["C_t"]
    icols_tb = layout["icols_tb"]
    chunk_off_in_tile = layout["chunk_off_in_tile"]
    icol_off_in_tile = layout["icol_off_in_tile"]
    ldns_col_off = layout["ldns_col_off"]
    icol_off_tile = layout["icol_off_tile"]
    Cmax = int(C_t.max())

    nc = bacc.Bacc(num_swdge_queues=4)
    feats = nc.declare_dram_parameter("feats", [n_nodes, P], f32, isOutput=False)
    idxm = nc.declare_dram_parameter(
        "idxm", [P, layout["total_icols"]], i16, isOutput=False
    )
    ldns = nc.declare_dram_parameter(
        "ldns", [P, layout["total_ldns"]], f32, isOutput=False
    )
    wt = nc.declare_dram_parameter("wt", [P, P], f32, isOutput=False)
    iotam = nc.declare_dram_parameter("iotam", [P, P], f32, isOutput=False)
    resid = nc.declare_dram_parameter("resid", [npc, P], f32, isOutput=False)
    out = nc.declare_dram_parameter("out", [npc, P], f32, isOutput=True)

    X_BUFS = 3
    with TileContext(nc) as tc:
        with (
            tc.tile_pool(name="const", bufs=1) as constp,
            tc.tile_pool(name="meta", bufs=3) as metap,
            tc.tile_pool(name="x", bufs=X_BUFS) as xp,
            tc.tile_pool(name="s", bufs=6) as sp,
            tc.tile_pool(name="zps", bufs=2, space="PSUM") as zpsp,
            tc.tile_pool(name="yps", bufs=2, space="PSUM") as ypsp,
            tc.tile_pool(name="post", bufs=3) as postp,
        ):
            wt_sb = constp.tile([P, P], f32)
            nc.sync.dma_start(out=wt_sb[:], in_=wt[:, :])
            iota_f = constp.tile([P, P], f32)
            nc.sync.dma_start(out=iota_f[:], in_=iotam[:, :])

            for t in range(n_tiles):
                Ct = int(C_t[t])
                icols = int(icols_tb[t].sum())
                mt_i = metap.tile([P, max(icols, 1)], i16, tag="mi")
                mt_ln = metap.tile([P, 3 * Ct], f32, tag="mldns")
                ic0 = int(icol_off_tile[t])
                nc.sync.dma_start(out=mt_i[:, :icols], in_=idxm[:, ic0 : ic0 + icols])
                lc0 = int(ldns_col_off[t])
                nc.sync.dma_start(out=mt_ln[:], in_=ldns[:, lc0 : lc0 + 3 * Ct])

                # X[e, c*128:(c+1)*128] = feats[gathered src of (chunk c, slot e)]
                X_full = xp.tile([P, Cmax * P], f32, tag="X")
                X = X_full[:, : Ct * P]
                for b in range(nb):
                    n_idx = int(n_tb[t, b])
                    if n_idx == 0:
                        continue
                    co = int(chunk_off_in_tile[t, b])
                    cb = int(ct_tb[t, b])
                    io = int(icol_off_in_tile[t, b])
                    icb = int(icols_tb[t, b])
                    if n_idx % P:
                        # the gather leaves partitions >= n_idx%128 of its
                        # last chunk unwritten; pre-zero that chunk so
                        # 0 * NaN can't poison the one-hot matmul (memzero
                        # bitcasts to uint32 - no NaN read path)
                        nc.scalar.memzero(X[:, (co + cb - 1) * P : (co + cb) * P])
                    nc.gpsimd.dma_gather(
                        out_ap=X[:, co * P : (co + cb) * P].rearrange(
                            "p (c e) -> p c e", e=P
                        ),
                        in_ap=feats[b * B : min((b + 1) * B, n_nodes), :],
                        idxs_ap=mt_i[:, io : io + icb],
                        num_idxs=n_idx,
                        num_idxs_reg=n_idx,
                        elem_size=P,
                        # single_packet concatenates the whole stream into one
                        # SDMA packet; the packet limit is 64 descriptors, and
                        # these calls emit ~70-90 per engine
                        single_packet=False,
                        # one SWDGE queue per bucket: queues run on distinct
                        # Q7 core pairs, parallelizing descriptor generation
                        queue_num=b % 4,
                    )

                z_ps = zpsp.tile([P, P], f32)
                for c in range(Ct):
                    S = sp.tile([P, P], f32, tag="S")
                    # split one-hot builds across DVE and ACT (nc.any piled
                    # all of them onto DVE: 2.9ms busy in the profile).
                    # ACT has no tensor_scalar; for integer iota/ld,
                    # relu(ns - ns*(ld-iota)^2) == (iota==ld)*ns exactly.
                    if c % 2 == 0:
                        nc.vector.tensor_scalar(
                            out=S[:],
                            in0=iota_f[:],
                            scalar1=mt_ln[:, c : c + 1],
                            scalar2=mt_ln[:, Ct + c : Ct + c + 1],
                            op0=mybir.AluOpType.is_equal,
                            op1=mybir.AluOpType.mult,
                        )
                    else:
                        t2 = sp.tile([P, P], f32, tag="T2")
                        nc.scalar.activation(
                            out=t2[:],
                            in_=iota_f[:],
                            func=mybir.ActivationFunctionType.Square,
                            bias=mt_ln[:, c : c + 1],
                            scale=-1.0,
                        )
                        nc.scalar.activation(
                            out=S[:],
                            in_=t2[:],
                            func=mybir.ActivationFunctionType.Relu,
                            bias=mt_ln[:, Ct + c : Ct + c + 1],
                            scale=mt_ln[:, 2 * Ct + c : 2 * Ct + c + 1],
                        )
                    # zT[i, d] += X_c[e, i].T @ S[e, d]
                    nc.tensor.matmul(
                        out=z_ps[:],
                        lhsT=X[:, c * P : (c + 1) * P],
                        rhs=S[:],
                        start=(c == 0),
                        stop=(c == Ct - 1),
                    )

                zT_sb = postp.tile([P, P], f32, tag="zT")
                nc.scalar.copy(out=zT_sb[:], in_=z_ps[:])
                y_ps = ypsp.tile([P, P], f32)
                # y[d, o] = zT[i, d].T @ wt[i, o]
                nc.tensor.matmul(
                    out=y_ps[:], lhsT=zT_sb[:], rhs=wt_sb[:], start=True, stop=True
                )

                rows = P if t < n_tiles - 1 else rows_last
                y_sb = postp.tile([P, P], f32, tag="y")
                nc.scalar.activation(
                    out=y_sb[:], in_=y_ps[:], func=mybir.ActivationFunctionType.Relu
                )
                res_sb = postp.tile([P, P], f32, tag="res")
                nc.sync.dma_start(
                    out=res_sb[:rows], in_=resid[t * P : t * P + rows, :]
                )
                o_sb = postp.tile([P, P], f32, tag="o")
                nc.vector.tensor_add(
                    out=o_sb[:rows], in0=y_sb[:rows], in1=res_sb[:rows]
                )
                nc.sync.dma_start(out=out[t * P : t * P + rows, :], in_=o_sb[:rows])
    nc.finalize()
    return nc


def _run(features, W, edge_src, edge_dst, trace=False, **spmd_kwargs):
    in_maps, layout = _prepare(features, W, edge_src, edge_dst)
    nc = _build_program(layout)
    br = run_bass_kernel_spmd(
        nc, in_maps, core_ids=list(range(N_CORES)), trace=trace, **spmd_kwargs
    )
    outs = [r["out"] for r in br.results]
    full = np.concatenate(outs, axis=0).astype(np.float32)
    return full, br


def kernel(features, W, edge_src, edge_dst):
    out, _ = _run(features, W, edge_src, edge_dst, trace=False)
    return out



# revision 7
# speedup vs baseline: 2.3262x; 2.3262x over previous
"""GCN layer (message passing) on 8 Trainium2 NeuronCores.

out = relu(((D^-1/2 A D^-1/2) X) @ W.T) + X

Strategy (dst-sharded graph partitioning, bf16 device path):
  - Destination nodes sharded across 8 cores (12500 nodes each); every core
    gathers from the full pre-normalized feature table h = X * D^-1/2 stored
    bf16 (256B rows -> half the HBM gather traffic of fp32); the host
    concatenates the 8 output slices.
  - Host-side prep (index-space only): edges grouped by (dst tile of 128
    nodes, src bucket of 25000 nodes, src ascending); per-(tile,bucket)
    gather sizes are max over the 8 cores (SPMD same-program constraint).
    Both degree norms leave the device inner loop: the src-side norm is
    folded into h, the dst-side norm is applied as a per-partition scale
    fused into the final ReLU.
  - Device, per dst tile t: 4 dma_gathers (one per src bucket, one SWDGE
    queue each) pull the edge source rows into X [128e x Ct*128] bf16. The
    one-hot scatter matrix S[e, (c,d)] = (ld[e,c] == d) for the whole tile
    is built by ONE DVE tensor_tensor(is_equal) against a constant iota-d
    plane, with ld broadcast along d via a stride-0 AP (replaces ~32
    tensor_scalar/activation ops of the fp32 version). Ct bf16 matmuls
    accumulate zT[f,d] in PSUM; zT -> bf16 SBUF (DVE copy);
    y[d,o] = zT.T @ W.T (PE); ReLU with per-partition scale norm[dst]
    (ACT); residual add (DVE); outputs staged in groups of 7 tiles and
    stored with one DMA per group (the fp32 version's per-tile small-row
    DMAs serialized ~1.6ms on one HWDGE queue).
  - X pool buffers are memzeroed once at start: gather tail slots that are
    never written afterwards only ever hold finite bf16 leftovers, which
    S=0 (ld=-1) annihilates without NaN risk.
"""

import math

import ml_dtypes
import numpy as np

import concourse.bacc as bacc
import concourse.mybir as mybir
from concourse.bass_utils import run_bass_kernel_spmd
from concourse.tile import TileContext

P = 128
N_CORES = 8
BUCKET_MAX = 25000  # int16 gather indices: bucket the node space
G_IO = 7  # tiles per residual-load/output-store group


def _prepare(features, W, edge_src, edge_dst, n_cores=N_CORES, bucket_max=BUCKET_MAX):
    """Partition the graph by dst core / dst tile / src bucket."""
    features = np.asarray(features, dtype=np.float32)
    W = np.asarray(W, dtype=np.float32)
    edge_src = np.asarray(edge_src, dtype=np.int32)
    edge_dst = np.asarray(edge_dst, dtype=np.int32)

    n_nodes, d = features.shape
    assert d == P
    assert n_nodes % n_cores == 0
    npc = n_nodes // n_cores
    n_tiles = math.ceil(npc / P)
    rows_last = npc - (n_tiles - 1) * P
    nb = math.ceil(n_nodes / bucket_max)
    B = math.ceil(n_nodes / nb)
    assert B <= 32768

    degs = np.bincount(edge_dst, minlength=n_nodes).astype(np.float32)
    norm = 1.0 / np.sqrt(np.maximum(degs, 1.0), dtype=np.float32)
    h16 = (features * norm[:, None]).astype(ml_dtypes.bfloat16)

    core_of = edge_dst // npc

    # first pass: per-core sorted edge lists and per-(tile,bucket) counts
    per_core = []
    counts_all = np.zeros((n_cores, n_tiles, nb), np.int64)
    for k in range(n_cores):
        sel = np.flatnonzero(core_of == k)
        src_k = edge_src[sel]
        ldst = edge_dst[sel] - k * npc
        tile_of = ldst // P
        bucket = src_k // B
        order = np.lexsort((src_k, bucket, tile_of))
        sel = sel[order]
        gid = tile_of[order] * nb + bucket[order]
        counts = np.bincount(gid, minlength=n_tiles * nb).reshape(n_tiles, nb)
        counts_all[k] = counts
        per_core.append((sel, gid, (ldst[order] % P).astype(np.float32)))

    # static per-(tile,bucket) gather sizes: max across cores
    n_tb = counts_all.max(axis=0)  # [n_tiles, nb]
    ct_tb = (n_tb + P - 1) // P  # chunks per (tile, bucket)
    C_t = ct_tb.sum(axis=1)  # chunks per tile
    Cmax = int(C_t.max())
    # gathers are padded to full 128-slot chunks (pad idx 0 -> gathers row 0
    # of the bucket, annihilated by ld=-1): every X slot is always written,
    # so no NaN can survive into the one-hot matmul
    icols_tb = ct_tb * (P // 16)  # int16 idx columns per (tile, bucket)
    icols_t = icols_tb.sum(axis=1)

    # column offsets in the packed DRAM arrays
    chunk_off_in_tile = np.cumsum(ct_tb, axis=1) - ct_tb  # [n_tiles, nb]
    icol_off_in_tile = np.cumsum(icols_tb, axis=1) - icols_tb
    C_off = np.concatenate([[0], np.cumsum(C_t)])[:-1]  # ld col offset per tile
    icol_off_tile = np.concatenate([[0], np.cumsum(icols_t)])[:-1]
    total_icols = int(icols_t.sum())
    total_C = int(C_t.sum())

    layout = dict(
        n_nodes=n_nodes,
        npc=npc,
        n_tiles=n_tiles,
        rows_last=rows_last,
        nb=nb,
        B=B,
        n_tb=n_tb,
        ct_tb=ct_tb,
        C_t=C_t,
        Cmax=Cmax,
        icols_tb=icols_tb,
        chunk_off_in_tile=chunk_off_in_tile,
        icol_off_in_tile=icol_off_in_tile,
        C_off=C_off,
        icol_off_tile=icol_off_tile,
        total_icols=total_icols,
        total_C=total_C,
    )

    # constant iota-d plane: col (c, d) -> d
    iota_d = np.tile(
        np.arange(P, dtype=np.float32).astype(ml_dtypes.bfloat16), (P, Cmax)
    )
    wt16 = np.ascontiguousarray(W.T).astype(ml_dtypes.bfloat16)  # wt[f, o] = W[o, f]

    in_maps = []
    for k in range(n_cores):
        sel, gid, ld_sorted = per_core[k]
        group_start = np.zeros(n_tiles * nb, np.int64)
        cnts = counts_all[k].reshape(-1)
        group_start[1:] = np.cumsum(cnts)[:-1]
        pos = np.arange(len(sel)) - group_start[gid]
        t_of = gid // nb
        b_of = gid % nb

        # idx array [16, total_icols] then replicated to 128 partitions
        idx16 = np.zeros((16, total_icols), np.int16)
        icol = icol_off_tile[t_of] + icol_off_in_tile[t_of, b_of] + pos // 16
        idx16[pos % 16, icol] = (edge_src[sel] - b_of * B).astype(np.int16)
        idxm = np.tile(idx16, (8, 1))

        # ld array [128, total_C] bf16: local dst slot per (chunk col, edge row)
        ld = np.full((P, total_C), -1.0, np.float32)
        cit = C_off[t_of] + chunk_off_in_tile[t_of, b_of] + pos // P
        ld[pos % P, cit] = ld_sorted
        ld16 = ld.astype(ml_dtypes.bfloat16)

        # per-partition dst norm per tile (1.0 on the unused tail rows)
        normT = np.ones((P, n_tiles), np.float32)
        ncol = norm[k * npc : (k + 1) * npc]
        nfull = (n_tiles - 1) * P
        normT[:, : n_tiles - 1] = ncol[:nfull].reshape(n_tiles - 1, P).T
        normT[:rows_last, n_tiles - 1] = ncol[nfull:]

        in_maps.append(
            {
                "feats": h16,
                "idxm": np.ascontiguousarray(idxm),
                "ld": np.ascontiguousarray(ld16),
                "wt": wt16,
                "iotad": iota_d,
                "normt": normT,
                "resid": np.ascontiguousarray(features[k * npc : (k + 1) * npc]),
            }
        )
    return in_maps, layout


def _build_program(layout):
    f32 = mybir.dt.float32
    bf16 = mybir.dt.bfloat16
    i16 = mybir.dt.int16
    n_nodes = layout["n_nodes"]
    npc = layout["npc"]
    n_tiles = layout["n_tiles"]
    rows_last = layout["rows_last"]
    nb = layout["nb"]
    B = layout["B"]
    n_tb = layout["n_tb"]
    ct_tb = layout["ct_tb"]
    C_t = layout["C_t"]
    Cmax = layout["Cmax"]
    icols_tb = layout["icols_tb"]
    chunk_off_in_tile = layout["chunk_off_in_tile"]
    icol_off_in_tile = layout["icol_off_in_tile"]
    C_off = layout["C_off"]
    icol_off_tile = layout["icol_off_tile"]
    total_icols = layout["total_icols"]
    total_C = layout["total_C"]

    nc = bacc.Bacc(num_swdge_queues=4)
    feats = nc.declare_dram_parameter("feats", [n_nodes, P], bf16, isOutput=False)
    idxm = nc.declare_dram_parameter("idxm", [P, total_icols], i16, isOutput=False)
    ldp = nc.declare_dram_parameter("ld", [P, total_C], bf16, isOutput=False)
    wt = nc.declare_dram_parameter("wt", [P, P], bf16, isOutput=False)
    iotad = nc.declare_dram_parameter("iotad", [P, Cmax * P], bf16, isOutput=False)
    normt = nc.declare_dram_parameter("normt", [P, n_tiles], f32, isOutput=False)
    resid = nc.declare_dram_parameter("resid", [npc, P], f32, isOutput=False)
    out = nc.declare_dram_parameter("out", [npc, P], f32, isOutput=True)

    n_groups = math.ceil(n_tiles / G_IO)
    X_BUFS = 4
    with TileContext(nc) as tc:
        with (
            tc.tile_pool(name="const", bufs=1) as constp,
            tc.tile_pool(name="x", bufs=X_BUFS) as xp,
            tc.tile_pool(name="s", bufs=3) as sp,
            tc.tile_pool(name="zps", bufs=2, space="PSUM") as zpsp,
            tc.tile_pool(name="yps", bufs=2, space="PSUM") as ypsp,
            tc.tile_pool(name="zt", bufs=3) as ztp,
            tc.tile_pool(name="y", bufs=3) as yp,
            tc.tile_pool(name="res", bufs=2) as resp,
            tc.tile_pool(name="og", bufs=2) as ogp,
        ):
            wt_sb = constp.tile([P, P], bf16)
            nc.sync.dma_start(out=wt_sb[:], in_=wt[:, :])
            iota_sb = constp.tile([P, Cmax * P], bf16)
            nc.sync.dma_start(out=iota_sb[:], in_=iotad[:, :])
            norm_sb = constp.tile([P, n_tiles], f32)
            nc.sync.dma_start(out=norm_sb[:], in_=normt[:, :])
            ld_sb = constp.tile([P, total_C], bf16)
            nc.sync.dma_start(out=ld_sb[:], in_=ldp[:, :])
            idx_sb = constp.tile([P, total_icols], i16)
            nc.sync.dma_start(out=idx_sb[:], in_=idxm[:, :])

            res_g = None
            og = None
            for t in range(n_tiles):
                g = t // G_IO
                j = t - g * G_IO
                if j == 0:
                    gt = min(G_IO, n_tiles - g * G_IO)
                    # tiles of this group that are full 128 rows
                    full_t = gt if g < n_groups - 1 or rows_last == P else gt - 1
                    res_g = resp.tile([P, G_IO * P], f32, tag="res")
                    og = ogp.tile([P, G_IO * P], f32, tag="og")
                    r0 = g * G_IO * P
                    if full_t:
                        nc.sync.dma_start(
                            out=res_g[:, : full_t * P].rearrange(
                                "p (t f) -> p t f", f=P
                            ),
                            in_=resid[r0 : r0 + full_t * P, :].rearrange(
                                "(t p) f -> p t f", p=P
                            ),
                        )
                    if full_t < gt:
                        nc.sync.dma_start(
                            out=res_g[:rows_last, full_t * P : (full_t + 1) * P],
                            in_=resid[r0 + full_t * P : npc, :],
                        )

                Ct = int(C_t[t])
                C0 = int(C_off[t])
                ic_t0 = int(icol_off_tile[t])

                # gather all edge source rows for this tile, one call per
                # src bucket on its own SWDGE queue
                X_full = xp.tile([P, Cmax * P], bf16, tag="X")
                X = X_full[:, : Ct * P]
                for b in range(nb):
                    n_idx = int(n_tb[t, b])
                    if n_idx == 0:
                        continue
                    co = int(chunk_off_in_tile[t, b])
                    cb = int(ct_tb[t, b])
                    io = ic_t0 + int(icol_off_in_tile[t, b])
                    icb = int(icols_tb[t, b])
                    nc.gpsimd.dma_gather(
                        out_ap=X[:, co * P : (co + cb) * P].rearrange(
                            "p (c e) -> p c e", e=P
                        ),
                        in_ap=feats[b * B : min((b + 1) * B, n_nodes), :],
                        idxs_ap=idx_sb[:, io : io + icb],
                        num_idxs=cb * P,
                        num_idxs_reg=cb * P,
                        elem_size=P,
                        single_packet=False,
                        queue_num=b % 4,
                    )

                # one-hot scatter matrix for the whole tile in ONE DVE op:
                # S[e, (c,d)] = (ld[e,c] == d), ld broadcast along d via
                # stride-0 AP
                S_full = sp.tile([P, Cmax * P], bf16, tag="S")
                S = S_full[:, : Ct * P]
                ld_b = (
                    ld_sb[:, C0 : C0 + Ct]
                    .rearrange("p (c u) -> p c u", u=1)
                    .broadcast_to([P, Ct, P])
                )
                nc.vector.tensor_tensor(
                    out=S.rearrange("p (c e) -> p c e", e=P),
                    in0=ld_b,
                    in1=iota_sb[:, : Ct * P].rearrange("p (c e) -> p c e", e=P),
                    op=mybir.AluOpType.is_equal,
                )

                # zT[f, d] += X_c[e, f].T @ S_c[e, d]
                z_ps = zpsp.tile([P, P], f32)
                for c in range(Ct):
                    nc.tensor.matmul(
                        out=z_ps[:],
                        lhsT=X[:, c * P : (c + 1) * P],
                        rhs=S[:, c * P : (c + 1) * P],
                        start=(c == 0),
                        stop=(c == Ct - 1),
                    )

                zT_sb = ztp.tile([P, P], bf16, tag="zT")
                nc.vector.tensor_copy(out=zT_sb[:], in_=z_ps[:])
                # y[d, o] = zT[f, d].T @ wt[f, o]
                y_ps = ypsp.tile([P, P], f32)
                nc.tensor.matmul(
                    out=y_ps[:], lhsT=zT_sb[:], rhs=wt_sb[:], start=True, stop=True
                )

                rows = P if t < n_tiles - 1 else rows_last
                # fused ReLU(y * norm[dst]) with per-partition scale
                y_sb = yp.tile([P, P], f32, tag="y")
                nc.scalar.activation(
                    out=y_sb[:],
                    in_=y_ps[:],
                    func=mybir.ActivationFunctionType.Relu,
                    scale=norm_sb[:, t : t + 1],
                )
                nc.vector.tensor_add(
                    out=og[:rows, j * P : (j + 1) * P],
                    in0=y_sb[:rows],
                    in1=res_g[:rows, j * P : (j + 1) * P],
                )

                if j == G_IO - 1 or t == n_tiles - 1:
                    gt = j + 1
                    full_t = gt if t < n_tiles - 1 or rows_last == P else gt - 1
                    r0 = g * G_IO * P
                    if full_t:
                        nc.sync.dma_start(
                            out=out[r0 : r0 + full_t * P, :].rearrange(
                                "(t p) f -> p t f", p=P
                            ),
                            in_=og[:, : full_t * P].rearrange(
                                "p (t f) -> p t f", f=P
                            ),
                        )
                    if full_t < gt:
                        nc.sync.dma_start(
                            out=out[r0 + full_t * P : npc, :],
                            in_=og[:rows_last, full_t * P : (full_t + 1) * P],
                        )
    nc.finalize()
    return nc


def _run(features, W, edge_src, edge_dst, trace=False, **spmd_kwargs):
    in_maps, layout = _prepare(features, W, edge_src, edge_dst)
    nc = _build_program(layout)
    br = run_bass_kernel_spmd(
        nc, in_maps, core_ids=list(range(N_CORES)), trace=trace, **spmd_kwargs
    )
    outs = [r["out"] for r in br.results]
    full = np.concatenate(outs, axis=0).astype(np.float32)
    return full, br


def kernel(features, W, edge_src, edge_dst):
    out, _ = _run(features, W, edge_src, edge_dst, trace=False)
    return out


# revision 11
# speedup vs baseline: 2.4361x; 1.0472x over previous
"""GCN layer (message passing) on 8 Trainium2 NeuronCores.

out = relu(((D^-1/2 A D^-1/2) X) @ W.T) + X

Strategy (dst-sharded graph partitioning, bf16 device path):
  - Destination nodes sharded across 8 cores (12500 nodes each); every core
    gathers from the full pre-normalized feature table h = X * D^-1/2 stored
    bf16 (256B rows -> half the HBM gather traffic of fp32); the host
    concatenates the 8 output slices.
  - Host-side prep (index-space only): edges grouped by (dst tile of 128
    nodes, src bucket of 25000 nodes, src ascending); per-(tile,bucket)
    gather sizes are max over the 8 cores (SPMD same-program constraint).
    Both degree norms leave the device inner loop: the src-side norm is
    folded into h, the dst-side norm is applied as a per-partition scale
    fused into the final ReLU.
  - Device, per dst tile t: 4 dma_gathers (one per src bucket, one SWDGE
    queue each) pull the edge source rows into X [128e x Ct*128] bf16. The
    one-hot scatter matrix S[e, (c,d)] = (ld[e,c] == d) for the whole tile
    is built by ONE DVE tensor_tensor(is_equal) against a constant iota-d
    plane, with ld broadcast along d via a stride-0 AP (replaces ~32
    tensor_scalar/activation ops of the fp32 version). Ct bf16 matmuls
    accumulate zT[f,d] in PSUM; zT -> bf16 SBUF (DVE copy);
    y[d,o] = zT.T @ W.T (PE); ReLU with per-partition scale norm[dst]
    (ACT); residual add (DVE); outputs staged in groups of 7 tiles and
    stored with one DMA per group (the fp32 version's per-tile small-row
    DMAs serialized ~1.6ms on one HWDGE queue).
  - X pool buffers are memzeroed once at start: gather tail slots that are
    never written afterwards only ever hold finite bf16 leftovers, which
    S=0 (ld=-1) annihilates without NaN risk.
"""

import math

import ml_dtypes
import numpy as np

import concourse.bacc as bacc
import concourse.mybir as mybir
from concourse.bass_utils import run_bass_kernel_spmd
from concourse.tile import TileContext

P = 128
N_CORES = 8
BUCKET_MAX = 25000  # int16 gather indices: bucket the node space
G_IO = 7  # tiles per residual-load/output-store group


def _prepare(features, W, edge_src, edge_dst, n_cores=N_CORES, bucket_max=BUCKET_MAX):
    """Partition the graph by dst core / dst tile / src bucket."""
    features = np.asarray(features, dtype=np.float32)
    W = np.asarray(W, dtype=np.float32)
    edge_src = np.asarray(edge_src, dtype=np.int32)
    edge_dst = np.asarray(edge_dst, dtype=np.int32)

    n_nodes, d = features.shape
    assert d == P
    assert n_nodes % n_cores == 0
    npc = n_nodes // n_cores
    n_tiles = math.ceil(npc / P)
    rows_last = npc - (n_tiles - 1) * P
    nb = math.ceil(n_nodes / bucket_max)
    B = math.ceil(n_nodes / nb)
    assert B <= 32768

    degs = np.bincount(edge_dst, minlength=n_nodes).astype(np.float32)
    norm = 1.0 / np.sqrt(np.maximum(degs, 1.0), dtype=np.float32)
    h16 = (features * norm[:, None]).astype(ml_dtypes.bfloat16)

    core_of = edge_dst // npc

    # first pass: per-core sorted edge lists and per-(tile,bucket) counts
    per_core = []
    counts_all = np.zeros((n_cores, n_tiles, nb), np.int64)
    for k in range(n_cores):
        sel = np.flatnonzero(core_of == k)
        src_k = edge_src[sel]
        ldst = edge_dst[sel] - k * npc
        tile_of = ldst // P
        bucket = src_k // B
        order = np.lexsort((src_k, bucket, tile_of))
        sel = sel[order]
        gid = tile_of[order] * nb + bucket[order]
        counts = np.bincount(gid, minlength=n_tiles * nb).reshape(n_tiles, nb)
        counts_all[k] = counts
        per_core.append((sel, gid, (ldst[order] % P).astype(np.float32)))

    # static per-(tile,bucket) gather sizes: max across cores
    n_tb = counts_all.max(axis=0)  # [n_tiles, nb]
    ct_tb = (n_tb + P - 1) // P  # chunks per (tile, bucket)
    C_t = ct_tb.sum(axis=1)  # chunks per tile
    Cmax = int(C_t.max())
    icols_tb = ct_tb * (P // 16)  # int16 idx columns per (tile, bucket)
    icols_t = icols_tb.sum(axis=1)

    # column offsets in the packed DRAM arrays
    chunk_off_in_tile = np.cumsum(ct_tb, axis=1) - ct_tb  # [n_tiles, nb]
    icol_off_in_tile = np.cumsum(icols_tb, axis=1) - icols_tb
    C_off = np.concatenate([[0], np.cumsum(C_t)])[:-1]  # ld col offset per tile
    icol_off_tile = np.concatenate([[0], np.cumsum(icols_t)])[:-1]
    total_icols = int(icols_t.sum())
    total_C = int(C_t.sum())

    layout = dict(
        n_nodes=n_nodes,
        npc=npc,
        n_tiles=n_tiles,
        rows_last=rows_last,
        nb=nb,
        B=B,
        n_tb=n_tb,
        ct_tb=ct_tb,
        C_t=C_t,
        Cmax=Cmax,
        icols_tb=icols_tb,
        chunk_off_in_tile=chunk_off_in_tile,
        icol_off_in_tile=icol_off_in_tile,
        C_off=C_off,
        icol_off_tile=icol_off_tile,
        total_icols=total_icols,
        total_C=total_C,
    )

    # constant iota-d plane: col (c, d) -> d
    iota_d = np.tile(
        np.arange(P, dtype=np.float32).astype(ml_dtypes.bfloat16), (P, Cmax)
    )
    wt16 = np.ascontiguousarray(W.T).astype(ml_dtypes.bfloat16)  # wt[f, o] = W[o, f]

    in_maps = []
    for k in range(n_cores):
        sel, gid, ld_sorted = per_core[k]
        group_start = np.zeros(n_tiles * nb, np.int64)
        cnts = counts_all[k].reshape(-1)
        group_start[1:] = np.cumsum(cnts)[:-1]
        pos = np.arange(len(sel)) - group_start[gid]
        t_of = gid // nb
        b_of = gid % nb

        # idx array [16, total_icols] then replicated to 128 partitions
        idx16 = np.zeros((16, total_icols), np.int16)
        icol = icol_off_tile[t_of] + icol_off_in_tile[t_of, b_of] + pos // 16
        idx16[pos % 16, icol] = (edge_src[sel] - b_of * B).astype(np.int16)
        idxm = np.tile(idx16, (8, 1))

        # ld array [128, total_C] bf16: local dst slot per (chunk col, edge row)
        ld = np.full((P, total_C), -1.0, np.float32)
        cit = C_off[t_of] + chunk_off_in_tile[t_of, b_of] + pos // P
        ld[pos % P, cit] = ld_sorted
        ld16 = ld.astype(ml_dtypes.bfloat16)

        # per-partition dst norm per tile (1.0 on the unused tail rows)
        normT = np.ones((P, n_tiles), np.float32)
        ncol = norm[k * npc : (k + 1) * npc]
        nfull = (n_tiles - 1) * P
        normT[:, : n_tiles - 1] = ncol[:nfull].reshape(n_tiles - 1, P).T
        normT[:rows_last, n_tiles - 1] = ncol[nfull:]

        in_maps.append(
            {
                "feats": h16,
                "idxm": np.ascontiguousarray(idxm),
                "ld": np.ascontiguousarray(ld16),
                "wt": wt16,
                "iotad": iota_d,
                "normt": normT,
                "resid": np.ascontiguousarray(features[k * npc : (k + 1) * npc]),
            }
        )
    return in_maps, layout


def _build_program(layout):
    f32 = mybir.dt.float32
    bf16 = mybir.dt.bfloat16
    i16 = mybir.dt.int16
    n_nodes = layout["n_nodes"]
    npc = layout["npc"]
    n_tiles = layout["n_tiles"]
    rows_last = layout["rows_last"]
    nb = layout["nb"]
    B = layout["B"]
    n_tb = layout["n_tb"]
    ct_tb = layout["ct_tb"]
    C_t = layout["C_t"]
    Cmax = layout["Cmax"]
    icols_tb = layout["icols_tb"]
    chunk_off_in_tile = layout["chunk_off_in_tile"]
    icol_off_in_tile = layout["icol_off_in_tile"]
    C_off = layout["C_off"]
    icol_off_tile = layout["icol_off_tile"]
    total_icols = layout["total_icols"]
    total_C = layout["total_C"]

    nc = bacc.Bacc(num_swdge_queues=4)
    feats = nc.declare_dram_parameter("feats", [n_nodes, P], bf16, isOutput=False)
    idxm = nc.declare_dram_parameter("idxm", [P, total_icols], i16, isOutput=False)
    ldp = nc.declare_dram_parameter("ld", [P, total_C], bf16, isOutput=False)
    wt = nc.declare_dram_parameter("wt", [P, P], bf16, isOutput=False)
    iotad = nc.declare_dram_parameter("iotad", [P, Cmax * P], bf16, isOutput=False)
    normt = nc.declare_dram_parameter("normt", [P, n_tiles], f32, isOutput=False)
    resid = nc.declare_dram_parameter("resid", [npc, P], f32, isOutput=False)
    out = nc.declare_dram_parameter("out", [npc, P], f32, isOutput=True)

    n_groups = math.ceil(n_tiles / G_IO)
    X_BUFS = 4
    with TileContext(nc) as tc:
        with (
            tc.tile_pool(name="const", bufs=1) as constp,
            tc.tile_pool(name="x", bufs=X_BUFS) as xp,
            tc.tile_pool(name="s", bufs=3) as sp,
            tc.tile_pool(name="zps", bufs=2, space="PSUM") as zpsp,
            tc.tile_pool(name="yps", bufs=2, space="PSUM") as ypsp,
            tc.tile_pool(name="zt", bufs=3) as ztp,
            tc.tile_pool(name="y", bufs=3) as yp,
            tc.tile_pool(name="res", bufs=2) as resp,
            tc.tile_pool(name="og", bufs=2) as ogp,
        ):
            wt_sb = constp.tile([P, P], bf16)
            nc.sync.dma_start(out=wt_sb[:], in_=wt[:, :])
            iota_sb = constp.tile([P, Cmax * P], bf16)
            nc.sync.dma_start(out=iota_sb[:], in_=iotad[:, :])
            norm_sb = constp.tile([P, n_tiles], f32)
            nc.sync.dma_start(out=norm_sb[:], in_=normt[:, :])
            ld_sb = constp.tile([P, total_C], bf16)
            nc.sync.dma_start(out=ld_sb[:], in_=ldp[:, :])
            # split the idx load so the first tiles' gathers can start
            # before the whole 6.4MB table lands (subtile deps)
            idx_sb = constp.tile([P, total_icols], i16)
            ic_head = int(icol_off_tile[min(8, n_tiles - 1)])
            nc.sync.dma_start(out=idx_sb[:, :ic_head], in_=idxm[:, :ic_head])
            nc.sync.dma_start(out=idx_sb[:, ic_head:], in_=idxm[:, ic_head:])

            res_g = None
            og = None
            for t in range(n_tiles):
                g = t // G_IO
                j = t - g * G_IO
                if j == 0:
                    gt = min(G_IO, n_tiles - g * G_IO)
                    # tiles of this group that are full 128 rows
                    full_t = gt if g < n_groups - 1 or rows_last == P else gt - 1
                    res_g = resp.tile([P, G_IO * P], f32, tag="res")
                    og = ogp.tile([P, G_IO * P], f32, tag="og")
                    r0 = g * G_IO * P
                    if full_t:
                        nc.sync.dma_start(
                            out=res_g[:, : full_t * P].rearrange(
                                "p (t f) -> p t f", f=P
                            ),
                            in_=resid[r0 : r0 + full_t * P, :].rearrange(
                                "(t p) f -> p t f", p=P
                            ),
                        )
                    if full_t < gt:
                        nc.sync.dma_start(
                            out=res_g[:rows_last, full_t * P : (full_t + 1) * P],
                            in_=resid[r0 + full_t * P : npc, :],
                        )

                Ct = int(C_t[t])
                C0 = int(C_off[t])
                ic_t0 = int(icol_off_tile[t])

                # gather all edge source rows for this tile, one call per
                # src bucket on its own SWDGE queue
                X_full = xp.tile([P, Cmax * P], bf16, tag="X")
                X = X_full[:, : Ct * P]
                for b in range(nb):
                    n_idx = int(n_tb[t, b])
                    if n_idx == 0:
                        continue
                    co = int(chunk_off_in_tile[t, b])
                    cb = int(ct_tb[t, b])
                    io = ic_t0 + int(icol_off_in_tile[t, b])
                    icb = (n_idx + 15) // 16
                    if n_idx % P:
                        # the gather leaves partitions >= n_idx%128 of its
                        # last chunk unwritten; pre-zero that chunk (on the
                        # ~idle ACT engine) so 0 * stale-NaN can't poison
                        # the one-hot matmul
                        nc.scalar.memzero(X[:, (co + cb - 1) * P : (co + cb) * P])
                    nc.gpsimd.dma_gather(
                        out_ap=X[:, co * P : (co + cb) * P].rearrange(
                            "p (c e) -> p c e", e=P
                        ),
                        in_ap=feats[b * B : min((b + 1) * B, n_nodes), :],
                        idxs_ap=idx_sb[:, io : io + icb],
                        num_idxs=n_idx,
                        num_idxs_reg=n_idx,
                        elem_size=P,
                        single_packet=False,
                        queue_num=b % 4,
                    )

                # one-hot scatter matrix for the whole tile in ONE DVE op:
                # S[e, (c,d)] = (ld[e,c] == d), ld broadcast along d via
                # stride-0 AP
                S_full = sp.tile([P, Cmax * P], bf16, tag="S")
                S = S_full[:, : Ct * P]
                ld_b = (
                    ld_sb[:, C0 : C0 + Ct]
                    .rearrange("p (c u) -> p c u", u=1)
                    .broadcast_to([P, Ct, P])
                )
                nc.vector.tensor_tensor(
                    out=S.rearrange("p (c e) -> p c e", e=P),
                    in0=ld_b,
                    in1=iota_sb[:, : Ct * P].rearrange("p (c e) -> p c e", e=P),
                    op=mybir.AluOpType.is_equal,
                )

                # zT[f, d] += X_c[e, f].T @ S_c[e, d]
                z_ps = zpsp.tile([P, P], f32)
                for c in range(Ct):
                    nc.tensor.matmul(
                        out=z_ps[:],
                        lhsT=X[:, c * P : (c + 1) * P],
                        rhs=S[:, c * P : (c + 1) * P],
                        start=(c == 0),
                        stop=(c == Ct - 1),
                    )

                zT_sb = ztp.tile([P, P], bf16, tag="zT")
                nc.vector.tensor_copy(out=zT_sb[:], in_=z_ps[:])
                # y[d, o] = zT[f, d].T @ wt[f, o]
                y_ps = ypsp.tile([P, P], f32)
                nc.tensor.matmul(
                    out=y_ps[:], lhsT=zT_sb[:], rhs=wt_sb[:], start=True, stop=True
                )

                rows = P if t < n_tiles - 1 else rows_last
                # fused ReLU(y * norm[dst]) with per-partition scale
                y_sb = yp.tile([P, P], f32, tag="y")
                nc.scalar.activation(
                    out=y_sb[:],
                    in_=y_ps[:],
                    func=mybir.ActivationFunctionType.Relu,
                    scale=norm_sb[:, t : t + 1],
                )
                nc.vector.tensor_add(
                    out=og[:rows, j * P : (j + 1) * P],
                    in0=y_sb[:rows],
                    in1=res_g[:rows, j * P : (j + 1) * P],
                )

                if j == G_IO - 1 or t == n_tiles - 1:
                    gt = j + 1
                    full_t = gt if t < n_tiles - 1 or rows_last == P else gt - 1
                    r0 = g * G_IO * P
                    if full_t:
                        nc.sync.dma_start(
                            out=out[r0 : r0 + full_t * P, :].rearrange(
                                "(t p) f -> p t f", p=P
                            ),
                            in_=og[:, : full_t * P].rearrange(
                                "p (t f) -> p t f", f=P
                            ),
                        )
                    if full_t < gt:
                        nc.sync.dma_start(
                            out=out[r0 + full_t * P : npc, :],
                            in_=og[:rows_last, full_t * P : (full_t + 1) * P],
                        )
    nc.finalize()
    return nc


def _run(features, W, edge_src, edge_dst, trace=False, **spmd_kwargs):
    in_maps, layout = _prepare(features, W, edge_src, edge_dst)
    nc = _build_program(layout)
    br = run_bass_kernel_spmd(
        nc, in_maps, core_ids=list(range(N_CORES)), trace=trace, **spmd_kwargs
    )
    outs = [r["out"] for r in br.results]
    full = np.concatenate(outs, axis=0).astype(np.float32)
    return full, br


def kernel(features, W, edge_src, edge_dst):
    out, _ = _run(features, W, edge_src, edge_dst, trace=False)
    return out


# revision 12
# speedup vs baseline: 2.7723x; 1.1380x over previous
"""GCN layer (message passing) on 8 Trainium2 NeuronCores.

out = relu(((D^-1/2 A D^-1/2) X) @ W.T) + X

Strategy (dst-sharded graph partitioning, bf16 device path):
  - Destination nodes sharded across 8 cores (12500 nodes each); every core
    gathers from the full pre-normalized feature table h = X * D^-1/2 stored
    bf16 (256B rows); the host concatenates the 8 output slices.
  - The SWDGE gather descriptor generation on the GpSimd Q7 pair is the
    bottleneck resource (~2.1ns/descriptor, and the ucode pads every call to
    full 128-slot chunks). To minimize descriptors, gathers are merged
    across quads of 4 dst tiles: one call per (quad, src bucket) -> 100
    calls instead of 392, and the chunk-rounding waste is paid once per
    quad instead of once per tile. Chunks on a tile boundary feed both
    tiles' accumulations (the one-hot matrices zero out foreign slots).
  - Both degree norms leave the device inner loop: the src-side norm is
    folded into h (host), the dst-side norm is applied as a per-partition
    scale fused into the final ReLU.
  - Device, per tile: the one-hot scatter matrix S[e, (j,d)] = (ld[e,j]==d)
    over the tile's chunk window is built by ONE DVE tensor_tensor
    (is_equal) against a constant iota-d plane, with ld broadcast along d
    via a stride-0 AP. W_t bf16 matmuls accumulate zT[f,d] in PSUM;
    zT -> bf16 SBUF (DVE copy); y = zT.T @ W.T (PE); ReLU with per-partition
    scale norm[dst] (ACT); residual add (DVE); outputs staged in groups of
    7 tiles and stored with one DMA per group.
  - Partial tail chunks of each gather are pre-zeroed on the ~idle ACT
    engine so 0 * stale-NaN cannot poison the PSUM accumulation; all other
    pad slots gather row 0 of the bucket and are killed by ld=-1.
"""

import math

import ml_dtypes
import numpy as np

import concourse.bacc as bacc
import concourse.mybir as mybir
from concourse.bass_utils import run_bass_kernel_spmd
from concourse.tile import TileContext

P = 128
N_CORES = 8
BUCKET_MAX = 25000  # int16 gather indices: bucket the node space
G_IO = 7  # tiles per residual-load/output-store group
G_Q = 4  # tiles per merged-gather quad


def _prepare(features, W, edge_src, edge_dst, n_cores=N_CORES, bucket_max=BUCKET_MAX):
    """Partition the graph by dst core / gather quad / src bucket."""
    features = np.asarray(features, dtype=np.float32)
    W = np.asarray(W, dtype=np.float32)
    edge_src = np.asarray(edge_src, dtype=np.int32)
    edge_dst = np.asarray(edge_dst, dtype=np.int32)

    n_nodes, d = features.shape
    assert d == P
    assert n_nodes % n_cores == 0
    npc = n_nodes // n_cores
    n_tiles = math.ceil(npc / P)
    rows_last = npc - (n_tiles - 1) * P
    nb = math.ceil(n_nodes / bucket_max)
    B = math.ceil(n_nodes / nb)
    assert B <= 32768

    degs = np.bincount(edge_dst, minlength=n_nodes).astype(np.float32)
    norm = 1.0 / np.sqrt(np.maximum(degs, 1.0), dtype=np.float32)
    h16 = (features * norm[:, None]).astype(ml_dtypes.bfloat16)

    core_of = edge_dst // npc

    # per-core sorted edge lists and per-(tile,bucket) counts
    per_core = []
    counts_all = np.zeros((n_cores, n_tiles, nb), np.int64)
    for k in range(n_cores):
        sel = np.flatnonzero(core_of == k)
        src_k = edge_src[sel]
        ldst = edge_dst[sel] - k * npc
        tile_of = ldst // P
        bucket = src_k // B
        order = np.lexsort((src_k, bucket, tile_of))
        sel = sel[order]
        gid = tile_of[order] * nb + bucket[order]
        counts = np.bincount(gid, minlength=n_tiles * nb).reshape(n_tiles, nb)
        counts_all[k] = counts
        per_core.append((sel, gid, (ldst[order] % P).astype(np.float32)))

    # static per-(tile,bucket) gather segment sizes: max across cores
    n_tb = counts_all.max(axis=0)  # [n_tiles, nb]

    # ---- static quad-merged gather layout ----
    nq = math.ceil(n_tiles / G_Q)
    quads = [(q * G_Q, min((q + 1) * G_Q, n_tiles)) for q in range(nq)]
    off_tb = np.zeros((n_tiles, nb), np.int64)  # slot offset in (q,b) stream
    L_qb = np.zeros((nq, nb), np.int64)  # stream length per (quad, bucket)
    CB_qb = np.zeros((nq, nb), np.int64)  # chunks per (quad, bucket)
    gc0_qb = np.zeros((nq, nb), np.int64)  # chunk base within quad X
    icol_qb = np.zeros((nq, nb), np.int64)  # idx col base (global)
    CQ_q = np.zeros(nq, np.int64)
    icol = 0
    for q, (t0, t1) in enumerate(quads):
        gc = 0
        for b in range(nb):
            off = 0
            for t in range(t0, t1):
                off_tb[t, b] = off
                off += n_tb[t, b]
            L_qb[q, b] = off
            CB_qb[q, b] = (off + P - 1) // P
            gc0_qb[q, b] = gc
            gc += CB_qb[q, b]
            icol_qb[q, b] = icol
            icol += (off + 15) // 16
        CQ_q[q] = gc
    total_icols = int(icol)
    CQmax = int(CQ_q.max())

    # per-tile chunk windows (within the quad X buffer) and ld columns
    chunk_map = []  # [t] -> list of quad-chunk indices
    wj0_tb = np.zeros((n_tiles, nb), np.int64)  # window start in chunk_map[t]
    for t in range(n_tiles):
        q = t // G_Q
        cm = []
        for b in range(nb):
            wj0_tb[t, b] = len(cm)
            n = n_tb[t, b]
            if n == 0:
                continue
            o = off_tb[t, b]
            c_lo = o // P
            c_hi = (o + n + P - 1) // P
            cm.extend(int(gc0_qb[q, b]) + c for c in range(c_lo, c_hi))
        chunk_map.append(cm)
    W_t = np.array([len(cm) for cm in chunk_map], np.int64)
    Wmax = int(W_t.max())
    L0_t = np.concatenate([[0], np.cumsum(W_t)])[:-1]
    total_L = int(W_t.sum())

    layout = dict(
        n_nodes=n_nodes,
        npc=npc,
        n_tiles=n_tiles,
        rows_last=rows_last,
        nb=nb,
        B=B,
        quads=quads,
        L_qb=L_qb,
        CB_qb=CB_qb,
        gc0_qb=gc0_qb,
        icol_qb=icol_qb,
        CQmax=CQmax,
        chunk_map=chunk_map,
        W_t=W_t,
        Wmax=Wmax,
        L0_t=L0_t,
        total_L=total_L,
        total_icols=total_icols,
    )

    # constant iota-d plane: col (j, d) -> d
    iota_d = np.tile(
        np.arange(P, dtype=np.float32).astype(ml_dtypes.bfloat16), (P, Wmax)
    )
    wt16 = np.ascontiguousarray(W.T).astype(ml_dtypes.bfloat16)  # wt[f, o] = W[o, f]

    in_maps = []
    for k in range(n_cores):
        sel, gid, ld_sorted = per_core[k]
        group_start = np.zeros(n_tiles * nb, np.int64)
        cnts = counts_all[k].reshape(-1)
        group_start[1:] = np.cumsum(cnts)[:-1]
        rank = np.arange(len(sel)) - group_start[gid]
        t_of = gid // nb
        b_of = gid % nb
        q_of = t_of // G_Q

        # stream position of each edge within its (quad, bucket) stream
        pos = off_tb[t_of, b_of] + rank

        # idx array [16, total_icols] then replicated to 128 partitions
        idx16 = np.zeros((16, total_icols), np.int16)
        ic = icol_qb[q_of, b_of] + pos // 16
        idx16[pos % 16, ic] = (edge_src[sel] - b_of * B).astype(np.int16)
        idxm = np.tile(idx16, (8, 1))

        # ld array [128, total_L] bf16: local dst per (tile window col, slot)
        ld = np.full((P, total_L), -1.0, np.float32)
        j = wj0_tb[t_of, b_of] + (pos // P - off_tb[t_of, b_of] // P)
        ld[pos % P, L0_t[t_of] + j] = ld_sorted
        ld16 = ld.astype(ml_dtypes.bfloat16)

        # per-partition dst norm per tile (1.0 on the unused tail rows)
        normT = np.ones((P, n_tiles), np.float32)
        ncol = norm[k * npc : (k + 1) * npc]
        nfull = (n_tiles - 1) * P
        normT[:, : n_tiles - 1] = ncol[:nfull].reshape(n_tiles - 1, P).T
        normT[:rows_last, n_tiles - 1] = ncol[nfull:]

        in_maps.append(
            {
                "feats": h16,
                "idxm": np.ascontiguousarray(idxm),
                "ld": np.ascontiguousarray(ld16),
                "wt": wt16,
                "iotad": iota_d,
                "normt": normT,
                "resid": np.ascontiguousarray(features[k * npc : (k + 1) * npc]),
            }
        )
    return in_maps, layout


def _build_program(layout):
    f32 = mybir.dt.float32
    bf16 = mybir.dt.bfloat16
    i16 = mybir.dt.int16
    n_nodes = layout["n_nodes"]
    npc = layout["npc"]
    n_tiles = layout["n_tiles"]
    rows_last = layout["rows_last"]
    nb = layout["nb"]
    B = layout["B"]
    quads = layout["quads"]
    L_qb = layout["L_qb"]
    CB_qb = layout["CB_qb"]
    gc0_qb = layout["gc0_qb"]
    icol_qb = layout["icol_qb"]
    CQmax = layout["CQmax"]
    chunk_map = layout["chunk_map"]
    W_t = layout["W_t"]
    Wmax = layout["Wmax"]
    L0_t = layout["L0_t"]
    total_L = layout["total_L"]
    total_icols = layout["total_icols"]

    nc = bacc.Bacc(num_swdge_queues=4)
    feats = nc.declare_dram_parameter("feats", [n_nodes, P], bf16, isOutput=False)
    idxm = nc.declare_dram_parameter("idxm", [P, total_icols], i16, isOutput=False)
    ldp = nc.declare_dram_parameter("ld", [P, total_L], bf16, isOutput=False)
    wt = nc.declare_dram_parameter("wt", [P, P], bf16, isOutput=False)
    iotad = nc.declare_dram_parameter("iotad", [P, Wmax * P], bf16, isOutput=False)
    normt = nc.declare_dram_parameter("normt", [P, n_tiles], f32, isOutput=False)
    resid = nc.declare_dram_parameter("resid", [npc, P], f32, isOutput=False)
    out = nc.declare_dram_parameter("out", [npc, P], f32, isOutput=True)

    n_groups = math.ceil(n_tiles / G_IO)
    with TileContext(nc) as tc:
        with (
            tc.tile_pool(name="const", bufs=1) as constp,
            tc.tile_pool(name="x", bufs=2) as xp,
            tc.tile_pool(name="s", bufs=3) as sp,
            tc.tile_pool(name="zps", bufs=2, space="PSUM") as zpsp,
            tc.tile_pool(name="yps", bufs=2, space="PSUM") as ypsp,
            tc.tile_pool(name="zt", bufs=3) as ztp,
            tc.tile_pool(name="y", bufs=3) as yp,
            tc.tile_pool(name="res", bufs=2) as resp,
            tc.tile_pool(name="og", bufs=2) as ogp,
        ):
            wt_sb = constp.tile([P, P], bf16)
            nc.sync.dma_start(out=wt_sb[:], in_=wt[:, :])
            iota_sb = constp.tile([P, Wmax * P], bf16)
            nc.sync.dma_start(out=iota_sb[:], in_=iotad[:, :])
            norm_sb = constp.tile([P, n_tiles], f32)
            nc.sync.dma_start(out=norm_sb[:], in_=normt[:, :])
            ld_sb = constp.tile([P, total_L], bf16)
            nc.sync.dma_start(out=ld_sb[:], in_=ldp[:, :])
            # split the idx load so the first quad's gathers can start
            # before the whole table lands (subtile deps)
            idx_sb = constp.tile([P, total_icols], i16)
            ic_head = int(icol_qb[1, 0]) if len(quads) > 1 else total_icols
            nc.sync.dma_start(out=idx_sb[:, :ic_head], in_=idxm[:, :ic_head])
            if ic_head < total_icols:
                nc.sync.dma_start(out=idx_sb[:, ic_head:], in_=idxm[:, ic_head:])

            res_g = None
            og = None
            for q, (t0, t1) in enumerate(quads):
                X = xp.tile([P, CQmax * P], bf16, tag="X")
                for b in range(nb):
                    L = int(L_qb[q, b])
                    if L == 0:
                        continue
                    cb = int(CB_qb[q, b])
                    g0 = int(gc0_qb[q, b])
                    io = int(icol_qb[q, b])
                    icb = (L + 15) // 16
                    if L % P:
                        # the gather leaves partitions >= L%128 of its last
                        # chunk unwritten; pre-zero that chunk (on the ~idle
                        # ACT engine) so 0 * stale-NaN can't poison the
                        # one-hot matmul
                        nc.scalar.memzero(X[:, (g0 + cb - 1) * P : (g0 + cb) * P])
                    nc.gpsimd.dma_gather(
                        out_ap=X[:, g0 * P : (g0 + cb) * P].rearrange(
                            "p (c e) -> p c e", e=P
                        ),
                        in_ap=feats[b * B : min((b + 1) * B, n_nodes), :],
                        idxs_ap=idx_sb[:, io : io + icb],
                        num_idxs=L,
                        num_idxs_reg=L,
                        elem_size=P,
                        single_packet=False,
                        queue_num=b % 4,
                    )

                for t in range(t0, t1):
                    g = t // G_IO
                    j = t - g * G_IO
                    if j == 0:
                        gt = min(G_IO, n_tiles - g * G_IO)
                        full_t = (
                            gt if g < n_groups - 1 or rows_last == P else gt - 1
                        )
                        res_g = resp.tile([P, G_IO * P], f32, tag="res")
                        og = ogp.tile([P, G_IO * P], f32, tag="og")
                        r0 = g * G_IO * P
                        if full_t:
                            nc.sync.dma_start(
                                out=res_g[:, : full_t * P].rearrange(
                                    "p (t f) -> p t f", f=P
                                ),
                                in_=resid[r0 : r0 + full_t * P, :].rearrange(
                                    "(t p) f -> p t f", p=P
                                ),
                            )
                        if full_t < gt:
                            nc.sync.dma_start(
                                out=res_g[:rows_last, full_t * P : (full_t + 1) * P],
                                in_=resid[r0 + full_t * P : npc, :],
                            )

                    Wt = int(W_t[t])
                    L0 = int(L0_t[t])

                    # one-hot scatter matrix for the whole tile in ONE DVE
                    # op: S[e, (j,d)] = (ld[e,j] == d), ld broadcast along d
                    # via stride-0 AP
                    S_full = sp.tile([P, Wmax * P], bf16, tag="S")
                    S = S_full[:, : Wt * P]
                    ld_b = (
                        ld_sb[:, L0 : L0 + Wt]
                        .rearrange("p (c u) -> p c u", u=1)
                        .broadcast_to([P, Wt, P])
                    )
                    nc.vector.tensor_tensor(
                        out=S.rearrange("p (c e) -> p c e", e=P),
                        in0=ld_b,
                        in1=iota_sb[:, : Wt * P].rearrange("p (c e) -> p c e", e=P),
                        op=mybir.AluOpType.is_equal,
                    )

                    # zT[f, d] += X_c[e, f].T @ S_j[e, d]
                    z_ps = zpsp.tile([P, P], f32)
                    for wj, gc in enumerate(chunk_map[t]):
                        nc.tensor.matmul(
                            out=z_ps[:],
                            lhsT=X[:, gc * P : (gc + 1) * P],
                            rhs=S[:, wj * P : (wj + 1) * P],
                            start=(wj == 0),
                            stop=(wj == Wt - 1),
                        )

                    zT_sb = ztp.tile([P, P], bf16, tag="zT")
                    nc.vector.tensor_copy(out=zT_sb[:], in_=z_ps[:])
                    # y[d, o] = zT[f, d].T @ wt[f, o]
                    y_ps = ypsp.tile([P, P], f32)
                    nc.tensor.matmul(
                        out=y_ps[:], lhsT=zT_sb[:], rhs=wt_sb[:], start=True, stop=True
                    )

                    rows = P if t < n_tiles - 1 else rows_last
                    # fused ReLU(y * norm[dst]) with per-partition scale
                    y_sb = yp.tile([P, P], f32, tag="y")
                    nc.scalar.activation(
                        out=y_sb[:],
                        in_=y_ps[:],
                        func=mybir.ActivationFunctionType.Relu,
                        scale=norm_sb[:, t : t + 1],
                    )
                    nc.vector.tensor_add(
                        out=og[:rows, j * P : (j + 1) * P],
                        in0=y_sb[:rows],
                        in1=res_g[:rows, j * P : (j + 1) * P],
                    )

                    if j == G_IO - 1 or t == n_tiles - 1:
                        gt = j + 1
                        full_t = (
                            gt if t < n_tiles - 1 or rows_last == P else gt - 1
                        )
                        r0 = g * G_IO * P
                        if full_t:
                            nc.sync.dma_start(
                                out=out[r0 : r0 + full_t * P, :].rearrange(
                                    "(t p) f -> p t f", p=P
                                ),
                                in_=og[:, : full_t * P].rearrange(
                                    "p (t f) -> p t f", f=P
                                ),
                            )
                        if full_t < gt:
                            nc.sync.dma_start(
                                out=out[r0 + full_t * P : npc, :],
                                in_=og[:rows_last, full_t * P : (full_t + 1) * P],
                            )
    nc.finalize()
    return nc


def _run(features, W, edge_src, edge_dst, trace=False, **spmd_kwargs):
    in_maps, layout = _prepare(features, W, edge_src, edge_dst)
    nc = _build_program(layout)
    br = run_bass_kernel_spmd(
        nc, in_maps, core_ids=list(range(N_CORES)), trace=trace, **spmd_kwargs
    )
    outs = [r["out"] for r in br.results]
    full = np.concatenate(outs, axis=0).astype(np.float32)
    return full, br


def kernel(features, W, edge_src, edge_dst):
    out, _ = _run(features, W, edge_src, edge_dst, trace=False)
    return out


# revision 15
# speedup vs baseline: 2.7793x; 1.0025x over previous
"""GCN layer (message passing) on 8 Trainium2 NeuronCores.

out = relu(((D^-1/2 A D^-1/2) X) @ W.T) + X

Strategy (dst-sharded graph partitioning, bf16 device path):
  - Destination nodes sharded across 8 cores (12500 nodes each); every core
    gathers from the full pre-normalized feature table h = X * D^-1/2 stored
    bf16 (256B rows); the host concatenates the 8 output slices.
  - The SWDGE gather descriptor generation on the GpSimd Q7 pair is the
    bottleneck resource (~2.1ns/descriptor, and the ucode pads every call to
    full 128-slot chunks). To minimize descriptors, gathers are merged
    across quads of 4 dst tiles: one call per (quad, src bucket) -> 100
    calls instead of 392, and the chunk-rounding waste is paid once per
    quad instead of once per tile. Chunks on a tile boundary feed both
    tiles' accumulations (the one-hot matrices zero out foreign slots).
  - Both degree norms leave the device inner loop: the src-side norm is
    folded into h (host), the dst-side norm is applied as a per-partition
    scale fused into the final ReLU.
  - Device, per tile: the one-hot scatter matrix S[e, (j,d)] = (ld[e,j]==d)
    over the tile's chunk window is built by ONE DVE tensor_tensor
    (is_equal) against a constant iota-d plane, with ld broadcast along d
    via a stride-0 AP. W_t bf16 matmuls accumulate zT[f,d] in PSUM;
    zT -> bf16 SBUF (DVE copy); y = zT.T @ W.T (PE); ReLU with per-partition
    scale norm[dst] (ACT); residual add (DVE); outputs staged in groups of
    7 tiles and stored with one DMA per group.
  - Partial tail chunks of each gather are pre-zeroed on the ~idle ACT
    engine so 0 * stale-NaN cannot poison the PSUM accumulation; all other
    pad slots gather row 0 of the bucket and are killed by ld=-1.
"""

import math

import ml_dtypes
import numpy as np

import concourse.bacc as bacc
import concourse.mybir as mybir
from concourse.bass_utils import run_bass_kernel_spmd
from concourse.tile import TileContext

P = 128
N_CORES = 8
BUCKET_MAX = 25000  # int16 gather indices: bucket the node space
G_IO = 7  # tiles per residual-load/output-store group
G_Q = 3  # tiles per merged-gather quad


def _prepare(features, W, edge_src, edge_dst, n_cores=N_CORES, bucket_max=BUCKET_MAX):
    """Partition the graph by dst core / gather quad / src bucket."""
    features = np.asarray(features, dtype=np.float32)
    W = np.asarray(W, dtype=np.float32)
    edge_src = np.asarray(edge_src, dtype=np.int32)
    edge_dst = np.asarray(edge_dst, dtype=np.int32)

    n_nodes, d = features.shape
    assert d == P
    assert n_nodes % n_cores == 0
    npc = n_nodes // n_cores
    n_tiles = math.ceil(npc / P)
    rows_last = npc - (n_tiles - 1) * P
    nb = math.ceil(n_nodes / bucket_max)
    B = math.ceil(n_nodes / nb)
    assert B <= 32768

    degs = np.bincount(edge_dst, minlength=n_nodes).astype(np.float32)
    norm = 1.0 / np.sqrt(np.maximum(degs, 1.0), dtype=np.float32)
    h16 = (features * norm[:, None]).astype(ml_dtypes.bfloat16)

    core_of = edge_dst // npc

    # per-core sorted edge lists and per-(tile,bucket) counts
    per_core = []
    counts_all = np.zeros((n_cores, n_tiles, nb), np.int64)
    for k in range(n_cores):
        sel = np.flatnonzero(core_of == k)
        src_k = edge_src[sel]
        ldst = edge_dst[sel] - k * npc
        tile_of = ldst // P
        bucket = src_k // B
        order = np.lexsort((src_k, bucket, tile_of))
        sel = sel[order]
        gid = tile_of[order] * nb + bucket[order]
        counts = np.bincount(gid, minlength=n_tiles * nb).reshape(n_tiles, nb)
        counts_all[k] = counts
        per_core.append((sel, gid, (ldst[order] % P).astype(np.float32)))

    # static per-(tile,bucket) gather segment sizes: max across cores
    n_tb = counts_all.max(axis=0)  # [n_tiles, nb]

    # ---- static quad-merged gather layout ----
    nq = math.ceil(n_tiles / G_Q)
    quads = [(q * G_Q, min((q + 1) * G_Q, n_tiles)) for q in range(nq)]
    off_tb = np.zeros((n_tiles, nb), np.int64)  # slot offset in (q,b) stream
    L_qb = np.zeros((nq, nb), np.int64)  # stream length per (quad, bucket)
    CB_qb = np.zeros((nq, nb), np.int64)  # chunks per (quad, bucket)
    gc0_qb = np.zeros((nq, nb), np.int64)  # chunk base within quad X
    icol_qb = np.zeros((nq, nb), np.int64)  # idx col base (global)
    CQ_q = np.zeros(nq, np.int64)
    icol = 0
    for q, (t0, t1) in enumerate(quads):
        gc = 0
        for b in range(nb):
            off = 0
            for t in range(t0, t1):
                off_tb[t, b] = off
                off += n_tb[t, b]
            L_qb[q, b] = off
            CB_qb[q, b] = (off + P - 1) // P
            gc0_qb[q, b] = gc
            gc += CB_qb[q, b]
            icol_qb[q, b] = icol
            icol += (off + 15) // 16
        CQ_q[q] = gc
    total_icols = int(icol)
    CQmax = int(CQ_q.max())

    # per-tile chunk windows (within the quad X buffer) and ld columns
    chunk_map = []  # [t] -> list of quad-chunk indices
    wj0_tb = np.zeros((n_tiles, nb), np.int64)  # window start in chunk_map[t]
    for t in range(n_tiles):
        q = t // G_Q
        cm = []
        for b in range(nb):
            wj0_tb[t, b] = len(cm)
            n = n_tb[t, b]
            if n == 0:
                continue
            o = off_tb[t, b]
            c_lo = o // P
            c_hi = (o + n + P - 1) // P
            cm.extend(int(gc0_qb[q, b]) + c for c in range(c_lo, c_hi))
        chunk_map.append(cm)
    W_t = np.array([len(cm) for cm in chunk_map], np.int64)
    Wmax = int(W_t.max())
    L0_t = np.concatenate([[0], np.cumsum(W_t)])[:-1]
    total_L = int(W_t.sum())

    layout = dict(
        n_nodes=n_nodes,
        npc=npc,
        n_tiles=n_tiles,
        rows_last=rows_last,
        nb=nb,
        B=B,
        quads=quads,
        L_qb=L_qb,
        CB_qb=CB_qb,
        gc0_qb=gc0_qb,
        icol_qb=icol_qb,
        CQmax=CQmax,
        chunk_map=chunk_map,
        W_t=W_t,
        Wmax=Wmax,
        L0_t=L0_t,
        total_L=total_L,
        total_icols=total_icols,
    )

    # constant iota-d plane: col (j, d) -> d
    iota_d = np.tile(
        np.arange(P, dtype=np.float32).astype(ml_dtypes.bfloat16), (P, Wmax)
    )
    wt16 = np.ascontiguousarray(W.T).astype(ml_dtypes.bfloat16)  # wt[f, o] = W[o, f]

    in_maps = []
    for k in range(n_cores):
        sel, gid, ld_sorted = per_core[k]
        group_start = np.zeros(n_tiles * nb, np.int64)
        cnts = counts_all[k].reshape(-1)
        group_start[1:] = np.cumsum(cnts)[:-1]
        rank = np.arange(len(sel)) - group_start[gid]
        t_of = gid // nb
        b_of = gid % nb
        q_of = t_of // G_Q

        # stream position of each edge within its (quad, bucket) stream
        pos = off_tb[t_of, b_of] + rank

        # idx array [16, total_icols] then replicated to 128 partitions
        idx16 = np.zeros((16, total_icols), np.int16)
        ic = icol_qb[q_of, b_of] + pos // 16
        idx16[pos % 16, ic] = (edge_src[sel] - b_of * B).astype(np.int16)
        idxm = np.tile(idx16, (8, 1))

        # ld array [128, total_L] bf16: local dst per (tile window col, slot)
        ld = np.full((P, total_L), -1.0, np.float32)
        j = wj0_tb[t_of, b_of] + (pos // P - off_tb[t_of, b_of] // P)
        ld[pos % P, L0_t[t_of] + j] = ld_sorted
        ld16 = ld.astype(ml_dtypes.bfloat16)

        # per-partition dst norm per tile (1.0 on the unused tail rows)
        normT = np.ones((P, n_tiles), np.float32)
        ncol = norm[k * npc : (k + 1) * npc]
        nfull = (n_tiles - 1) * P
        normT[:, : n_tiles - 1] = ncol[:nfull].reshape(n_tiles - 1, P).T
        normT[:rows_last, n_tiles - 1] = ncol[nfull:]

        in_maps.append(
            {
                "feats": h16,
                "idxm": np.ascontiguousarray(idxm),
                "ld": np.ascontiguousarray(ld16),
                "wt": wt16,
                "iotad": iota_d,
                "normt": normT,
                "resid": np.ascontiguousarray(features[k * npc : (k + 1) * npc]),
            }
        )
    return in_maps, layout


def _build_program(layout):
    f32 = mybir.dt.float32
    bf16 = mybir.dt.bfloat16
    i16 = mybir.dt.int16
    n_nodes = layout["n_nodes"]
    npc = layout["npc"]
    n_tiles = layout["n_tiles"]
    rows_last = layout["rows_last"]
    nb = layout["nb"]
    B = layout["B"]
    quads = layout["quads"]
    L_qb = layout["L_qb"]
    CB_qb = layout["CB_qb"]
    gc0_qb = layout["gc0_qb"]
    icol_qb = layout["icol_qb"]
    CQmax = layout["CQmax"]
    chunk_map = layout["chunk_map"]
    W_t = layout["W_t"]
    Wmax = layout["Wmax"]
    L0_t = layout["L0_t"]
    total_L = layout["total_L"]
    total_icols = layout["total_icols"]

    nc = bacc.Bacc(num_swdge_queues=4)
    feats = nc.declare_dram_parameter("feats", [n_nodes, P], bf16, isOutput=False)
    idxm = nc.declare_dram_parameter("idxm", [P, total_icols], i16, isOutput=False)
    ldp = nc.declare_dram_parameter("ld", [P, total_L], bf16, isOutput=False)
    wt = nc.declare_dram_parameter("wt", [P, P], bf16, isOutput=False)
    iotad = nc.declare_dram_parameter("iotad", [P, Wmax * P], bf16, isOutput=False)
    normt = nc.declare_dram_parameter("normt", [P, n_tiles], f32, isOutput=False)
    resid = nc.declare_dram_parameter("resid", [npc, P], f32, isOutput=False)
    out = nc.declare_dram_parameter("out", [npc, P], f32, isOutput=True)

    n_groups = math.ceil(n_tiles / G_IO)
    with TileContext(nc) as tc:
        with (
            tc.tile_pool(name="const", bufs=1) as constp,
            tc.tile_pool(name="x", bufs=3) as xp,
            tc.tile_pool(name="s", bufs=3) as sp,
            tc.tile_pool(name="zps", bufs=2, space="PSUM") as zpsp,
            tc.tile_pool(name="yps", bufs=2, space="PSUM") as ypsp,
            tc.tile_pool(name="zt", bufs=3) as ztp,
            tc.tile_pool(name="y", bufs=3) as yp,
            tc.tile_pool(name="res", bufs=2) as resp,
            tc.tile_pool(name="og", bufs=2) as ogp,
        ):
            # idx head first: the first quads' gathers depend only on it
            # (subtile deps), everything else can land behind it
            idx_sb = constp.tile([P, total_icols], i16)
            ic_head = int(icol_qb[2, 0]) if len(quads) > 2 else total_icols
            nc.sync.dma_start(out=idx_sb[:, :ic_head], in_=idxm[:, :ic_head])
            ld_sb = constp.tile([P, total_L], bf16)
            nc.sync.dma_start(out=ld_sb[:], in_=ldp[:, :])
            iota_sb = constp.tile([P, Wmax * P], bf16)
            nc.sync.dma_start(out=iota_sb[:], in_=iotad[:, :])
            wt_sb = constp.tile([P, P], bf16)
            nc.sync.dma_start(out=wt_sb[:], in_=wt[:, :])
            norm_sb = constp.tile([P, n_tiles], f32)
            nc.sync.dma_start(out=norm_sb[:], in_=normt[:, :])
            if ic_head < total_icols:
                nc.sync.dma_start(out=idx_sb[:, ic_head:], in_=idxm[:, ic_head:])

            res_g = None
            og = None
            for q, (t0, t1) in enumerate(quads):
                X = xp.tile([P, CQmax * P], bf16, tag="X")
                for b in range(nb):
                    L = int(L_qb[q, b])
                    if L == 0:
                        continue
                    cb = int(CB_qb[q, b])
                    g0 = int(gc0_qb[q, b])
                    io = int(icol_qb[q, b])
                    icb = (L + 15) // 16
                    if L % P:
                        # the gather leaves partitions >= L%128 of its last
                        # chunk unwritten; pre-zero that chunk (on the ~idle
                        # ACT engine) so 0 * stale-NaN can't poison the
                        # one-hot matmul
                        nc.scalar.memzero(X[:, (g0 + cb - 1) * P : (g0 + cb) * P])
                    nc.gpsimd.dma_gather(
                        out_ap=X[:, g0 * P : (g0 + cb) * P].rearrange(
                            "p (c e) -> p c e", e=P
                        ),
                        in_ap=feats[b * B : min((b + 1) * B, n_nodes), :],
                        idxs_ap=idx_sb[:, io : io + icb],
                        num_idxs=L,
                        num_idxs_reg=L,
                        elem_size=P,
                        single_packet=False,
                        queue_num=b % 4,
                    )

                for t in range(t0, t1):
                    g = t // G_IO
                    j = t - g * G_IO
                    if j == 0:
                        gt = min(G_IO, n_tiles - g * G_IO)
                        full_t = (
                            gt if g < n_groups - 1 or rows_last == P else gt - 1
                        )
                        res_g = resp.tile([P, G_IO * P], f32, tag="res")
                        og = ogp.tile([P, G_IO * P], f32, tag="og")
                        r0 = g * G_IO * P
                        if full_t:
                            nc.sync.dma_start(
                                out=res_g[:, : full_t * P].rearrange(
                                    "p (t f) -> p t f", f=P
                                ),
                                in_=resid[r0 : r0 + full_t * P, :].rearrange(
                                    "(t p) f -> p t f", p=P
                                ),
                            )
                        if full_t < gt:
                            nc.sync.dma_start(
                                out=res_g[:rows_last, full_t * P : (full_t + 1) * P],
                                in_=resid[r0 + full_t * P : npc, :],
                            )

                    Wt = int(W_t[t])
                    L0 = int(L0_t[t])

                    # one-hot scatter matrix for the whole tile in ONE DVE
                    # op: S[e, (j,d)] = (ld[e,j] == d), ld broadcast along d
                    # via stride-0 AP
                    S_full = sp.tile([P, Wmax * P], bf16, tag="S")
                    S = S_full[:, : Wt * P]
                    ld_b = (
                        ld_sb[:, L0 : L0 + Wt]
                        .rearrange("p (c u) -> p c u", u=1)
                        .broadcast_to([P, Wt, P])
                    )
                    nc.vector.tensor_tensor(
                        out=S.rearrange("p (c e) -> p c e", e=P),
                        in0=ld_b,
                        in1=iota_sb[:, : Wt * P].rearrange("p (c e) -> p c e", e=P),
                        op=mybir.AluOpType.is_equal,
                    )

                    # zT[f, d] += X_c[e, f].T @ S_j[e, d]
                    z_ps = zpsp.tile([P, P], f32)
                    for wj, gc in enumerate(chunk_map[t]):
                        nc.tensor.matmul(
                            out=z_ps[:],
                            lhsT=X[:, gc * P : (gc + 1) * P],
                            rhs=S[:, wj * P : (wj + 1) * P],
                            start=(wj == 0),
                            stop=(wj == Wt - 1),
                        )

                    zT_sb = ztp.tile([P, P], bf16, tag="zT")
                    nc.vector.tensor_copy(out=zT_sb[:], in_=z_ps[:])
                    # y[d, o] = zT[f, d].T @ wt[f, o]
                    y_ps = ypsp.tile([P, P], f32)
                    nc.tensor.matmul(
                        out=y_ps[:], lhsT=zT_sb[:], rhs=wt_sb[:], start=True, stop=True
                    )

                    rows = P if t < n_tiles - 1 else rows_last
                    # fused ReLU(y * norm[dst]) with per-partition scale
                    y_sb = yp.tile([P, P], f32, tag="y")
                    nc.scalar.activation(
                        out=y_sb[:],
                        in_=y_ps[:],
                        func=mybir.ActivationFunctionType.Relu,
                        scale=norm_sb[:, t : t + 1],
                    )
                    nc.vector.tensor_add(
                        out=og[:rows, j * P : (j + 1) * P],
                        in0=y_sb[:rows],
                        in1=res_g[:rows, j * P : (j + 1) * P],
                    )

                    if j == G_IO - 1 or t == n_tiles - 1:
                        gt = j + 1
                        full_t = (
                            gt if t < n_tiles - 1 or rows_last == P else gt - 1
                        )
                        r0 = g * G_IO * P
                        if full_t:
                            nc.sync.dma_start(
                                out=out[r0 : r0 + full_t * P, :].rearrange(
                                    "(t p) f -> p t f", p=P
                                ),
                                in_=og[:, : full_t * P].rearrange(
                                    "p (t f) -> p t f", f=P
                                ),
                            )
                        if full_t < gt:
                            nc.sync.dma_start(
                                out=out[r0 + full_t * P : npc, :],
                                in_=og[:rows_last, full_t * P : (full_t + 1) * P],
                            )
    nc.finalize()
    return nc


def _run(features, W, edge_src, edge_dst, trace=False, **spmd_kwargs):
    in_maps, layout = _prepare(features, W, edge_src, edge_dst)
    nc = _build_program(layout)
    br = run_bass_kernel_spmd(
        nc, in_maps, core_ids=list(range(N_CORES)), trace=trace, **spmd_kwargs
    )
    outs = [r["out"] for r in br.results]
    full = np.concatenate(outs, axis=0).astype(np.float32)
    return full, br


def kernel(features, W, edge_src, edge_dst):
    out, _ = _run(features, W, edge_src, edge_dst, trace=False)
    return out


# revision 23
# speedup vs baseline: 2.8096x; 1.0109x over previous
"""GCN layer (message passing) on 8 Trainium2 NeuronCores.

out = relu(((D^-1/2 A D^-1/2) X) @ W.T) + X

Strategy (dst-sharded graph partitioning, bf16 device path):
  - Destination nodes sharded across 8 cores (12500 nodes each); every core
    gathers from the full pre-normalized feature table h = X * D^-1/2 stored
    bf16 (256B rows); the host concatenates the 8 output slices.
  - The SWDGE gather descriptor generation on the GpSimd Q7 pair is the
    bottleneck resource (~2.1ns/descriptor, and the ucode pads every call to
    full 128-slot chunks). To minimize descriptors, gathers are merged
    across quads of 4 dst tiles: one call per (quad, src bucket) -> 100
    calls instead of 392, and the chunk-rounding waste is paid once per
    quad instead of once per tile. Chunks on a tile boundary feed both
    tiles' accumulations (the one-hot matrices zero out foreign slots).
  - Both degree norms leave the device inner loop: the src-side norm is
    folded into h (host), the dst-side norm is applied as a per-partition
    scale fused into the final ReLU.
  - Device, per tile: the one-hot scatter matrix S[e, (j,d)] = (ld[e,j]==d)
    over the tile's chunk window is built by ONE DVE tensor_tensor
    (is_equal) against a constant iota-d plane, with ld broadcast along d
    via a stride-0 AP. W_t bf16 matmuls accumulate zT[f,d] in PSUM;
    zT -> bf16 SBUF (DVE copy); y = zT.T @ W.T (PE); ReLU with per-partition
    scale norm[dst] (ACT); residual add (DVE); outputs staged in groups of
    7 tiles and stored with one DMA per group.
  - Partial tail chunks of each gather are pre-zeroed on the ~idle ACT
    engine so 0 * stale-NaN cannot poison the PSUM accumulation; all other
    pad slots gather row 0 of the bucket and are killed by ld=-1.
"""

import math

import ml_dtypes
import numpy as np

import concourse.bacc as bacc
import concourse.mybir as mybir
from concourse.bass_utils import run_bass_kernel_spmd
from concourse.tile import TileContext

P = 128
N_CORES = 8
BUCKET_MAX = 25000  # int16 gather indices: bucket the node space
G_IO = 7  # tiles per residual-load/output-store group
G_Q = 4  # tiles per merged-gather quad
G_IDX = 8  # quads per staged idx-table load


def _prepare(features, W, edge_src, edge_dst, n_cores=N_CORES, bucket_max=BUCKET_MAX):
    """Partition the graph by dst core / gather quad / src bucket."""
    features = np.asarray(features, dtype=np.float32)
    W = np.asarray(W, dtype=np.float32)
    edge_src = np.asarray(edge_src, dtype=np.int32)
    edge_dst = np.asarray(edge_dst, dtype=np.int32)

    n_nodes, d = features.shape
    assert d == P
    assert n_nodes % n_cores == 0
    npc = n_nodes // n_cores
    n_tiles = math.ceil(npc / P)
    rows_last = npc - (n_tiles - 1) * P
    nb = math.ceil(n_nodes / bucket_max)
    B = math.ceil(n_nodes / nb)
    assert B <= 32768

    degs = np.bincount(edge_dst, minlength=n_nodes).astype(np.float32)
    norm = 1.0 / np.sqrt(np.maximum(degs, 1.0), dtype=np.float32)
    h16 = (features * norm[:, None]).astype(ml_dtypes.bfloat16)

    core_of = edge_dst // npc

    # per-core sorted edge lists and per-(tile,bucket) counts
    per_core = []
    counts_all = np.zeros((n_cores, n_tiles, nb), np.int64)
    for k in range(n_cores):
        sel = np.flatnonzero(core_of == k)
        src_k = edge_src[sel]
        ldst = edge_dst[sel] - k * npc
        tile_of = ldst // P
        bucket = src_k // B
        order = np.lexsort((src_k, bucket, tile_of))
        sel = sel[order]
        gid = tile_of[order] * nb + bucket[order]
        counts = np.bincount(gid, minlength=n_tiles * nb).reshape(n_tiles, nb)
        counts_all[k] = counts
        per_core.append((sel, gid, (ldst[order] % P).astype(np.float32)))

    # static per-(tile,bucket) gather segment sizes: max across cores
    n_tb = counts_all.max(axis=0)  # [n_tiles, nb]

    # ---- static quad-merged gather layout ----
    nq = math.ceil(n_tiles / G_Q)
    quads = [(q * G_Q, min((q + 1) * G_Q, n_tiles)) for q in range(nq)]
    off_tb = np.zeros((n_tiles, nb), np.int64)  # slot offset in (q,b) stream
    L_qb = np.zeros((nq, nb), np.int64)  # stream length per (quad, bucket)
    CB_qb = np.zeros((nq, nb), np.int64)  # chunks per (quad, bucket)
    gc0_qb = np.zeros((nq, nb), np.int64)  # chunk base within quad X
    icol_qb = np.zeros((nq, nb), np.int64)  # idx col base (global)
    CQ_q = np.zeros(nq, np.int64)
    icol = 0
    for q, (t0, t1) in enumerate(quads):
        gc = 0
        for b in range(nb):
            off = 0
            for t in range(t0, t1):
                off_tb[t, b] = off
                off += n_tb[t, b]
            L_qb[q, b] = off
            CB_qb[q, b] = (off + P - 1) // P
            gc0_qb[q, b] = gc
            gc += CB_qb[q, b]
            icol_qb[q, b] = icol
            icol += (off + 15) // 16
        CQ_q[q] = gc
    total_icols = int(icol)
    CQmax = int(CQ_q.max())

    # idx table is staged in groups of G_IDX quads (keeping it fully
    # resident would crowd out the third X buffer)
    n_ig = math.ceil(nq / G_IDX)
    ig_col0 = np.zeros(n_ig + 1, np.int64)
    for g in range(n_ig):
        q_end = min((g + 1) * G_IDX, nq)
        ig_col0[g + 1] = (
            int(icol_qb[q_end, 0]) if q_end < nq else total_icols
        )
    ICGmax = int(np.diff(ig_col0).max()) if n_ig else 0

    # per-tile chunk windows (within the quad X buffer) and ld columns
    chunk_map = []  # [t] -> list of quad-chunk indices
    wj0_tb = np.zeros((n_tiles, nb), np.int64)  # window start in chunk_map[t]
    for t in range(n_tiles):
        q = t // G_Q
        cm = []
        for b in range(nb):
            wj0_tb[t, b] = len(cm)
            n = n_tb[t, b]
            if n == 0:
                continue
            o = off_tb[t, b]
            c_lo = o // P
            c_hi = (o + n + P - 1) // P
            cm.extend(int(gc0_qb[q, b]) + c for c in range(c_lo, c_hi))
        chunk_map.append(cm)
    W_t = np.array([len(cm) for cm in chunk_map], np.int64)
    Wmax = int(W_t.max())
    L0_t = np.concatenate([[0], np.cumsum(W_t)])[:-1]
    total_L = int(W_t.sum())

    layout = dict(
        n_nodes=n_nodes,
        npc=npc,
        n_tiles=n_tiles,
        rows_last=rows_last,
        nb=nb,
        B=B,
        quads=quads,
        L_qb=L_qb,
        CB_qb=CB_qb,
        gc0_qb=gc0_qb,
        icol_qb=icol_qb,
        CQmax=CQmax,
        ig_col0=ig_col0,
        ICGmax=ICGmax,
        chunk_map=chunk_map,
        W_t=W_t,
        Wmax=Wmax,
        L0_t=L0_t,
        total_L=total_L,
        total_icols=total_icols,
    )

    # constant iota-d plane: col (j, d) -> d
    iota_d = np.tile(
        np.arange(P, dtype=np.float32).astype(ml_dtypes.bfloat16), (P, Wmax)
    )
    wt16 = np.ascontiguousarray(W.T).astype(ml_dtypes.bfloat16)  # wt[f, o] = W[o, f]

    in_maps = []
    for k in range(n_cores):
        sel, gid, ld_sorted = per_core[k]
        group_start = np.zeros(n_tiles * nb, np.int64)
        cnts = counts_all[k].reshape(-1)
        group_start[1:] = np.cumsum(cnts)[:-1]
        rank = np.arange(len(sel)) - group_start[gid]
        t_of = gid // nb
        b_of = gid % nb
        q_of = t_of // G_Q

        # stream position of each edge within its (quad, bucket) stream
        pos = off_tb[t_of, b_of] + rank

        # idx array [16, total_icols] then replicated to 128 partitions
        idx16 = np.zeros((16, total_icols), np.int16)
        ic = icol_qb[q_of, b_of] + pos // 16
        idx16[pos % 16, ic] = (edge_src[sel] - b_of * B).astype(np.int16)
        idxm = np.tile(idx16, (8, 1))

        # ld array [128, total_L] bf16: local dst per (tile window col, slot)
        ld = np.full((P, total_L), -1.0, np.float32)
        j = wj0_tb[t_of, b_of] + (pos // P - off_tb[t_of, b_of] // P)
        ld[pos % P, L0_t[t_of] + j] = ld_sorted
        ld16 = ld.astype(ml_dtypes.bfloat16)

        # per-partition dst norm per tile (1.0 on the unused tail rows)
        normT = np.ones((P, n_tiles), np.float32)
        ncol = norm[k * npc : (k + 1) * npc]
        nfull = (n_tiles - 1) * P
        normT[:, : n_tiles - 1] = ncol[:nfull].reshape(n_tiles - 1, P).T
        normT[:rows_last, n_tiles - 1] = ncol[nfull:]

        in_maps.append(
            {
                "feats": h16,
                "idxm": np.ascontiguousarray(idxm),
                "ld": np.ascontiguousarray(ld16),
                "wt": wt16,
                "iotad": iota_d,
                "normt": normT,
                "resid": np.ascontiguousarray(features[k * npc : (k + 1) * npc]),
            }
        )
    return in_maps, layout


def _build_program(layout):
    f32 = mybir.dt.float32
    bf16 = mybir.dt.bfloat16
    i16 = mybir.dt.int16
    n_nodes = layout["n_nodes"]
    npc = layout["npc"]
    n_tiles = layout["n_tiles"]
    rows_last = layout["rows_last"]
    nb = layout["nb"]
    B = layout["B"]
    quads = layout["quads"]
    L_qb = layout["L_qb"]
    CB_qb = layout["CB_qb"]
    gc0_qb = layout["gc0_qb"]
    icol_qb = layout["icol_qb"]
    CQmax = layout["CQmax"]
    ig_col0 = layout["ig_col0"]
    ICGmax = layout["ICGmax"]
    chunk_map = layout["chunk_map"]
    W_t = layout["W_t"]
    Wmax = layout["Wmax"]
    L0_t = layout["L0_t"]
    total_L = layout["total_L"]
    total_icols = layout["total_icols"]

    nc = bacc.Bacc(num_swdge_queues=4)
    feats = nc.declare_dram_parameter("feats", [n_nodes, P], bf16, isOutput=False)
    idxm = nc.declare_dram_parameter("idxm", [P, total_icols], i16, isOutput=False)
    ldp = nc.declare_dram_parameter("ld", [P, total_L], bf16, isOutput=False)
    wt = nc.declare_dram_parameter("wt", [P, P], bf16, isOutput=False)
    iotad = nc.declare_dram_parameter("iotad", [P, Wmax * P], bf16, isOutput=False)
    normt = nc.declare_dram_parameter("normt", [P, n_tiles], f32, isOutput=False)
    resid = nc.declare_dram_parameter("resid", [npc, P], f32, isOutput=False)
    out = nc.declare_dram_parameter("out", [npc, P], f32, isOutput=True)

    n_groups = math.ceil(n_tiles / G_IO)
    with TileContext(nc) as tc:
        with (
            tc.tile_pool(name="const", bufs=1) as constp,
            tc.tile_pool(name="idx", bufs=3) as idxp,
            tc.tile_pool(name="x", bufs=3) as xp,
            tc.tile_pool(name="s", bufs=2) as sp,
            tc.tile_pool(name="zps", bufs=2, space="PSUM") as zpsp,
            tc.tile_pool(name="yps", bufs=2, space="PSUM") as ypsp,
            tc.tile_pool(name="zt", bufs=3) as ztp,
            tc.tile_pool(name="y", bufs=3) as yp,
            tc.tile_pool(name="res", bufs=2) as resp,
            tc.tile_pool(name="og", bufs=2) as ogp,
        ):
            ld_sb = constp.tile([P, total_L], bf16)
            nc.sync.dma_start(out=ld_sb[:], in_=ldp[:, :])
            iota_sb = constp.tile([P, Wmax * P], bf16)
            nc.sync.dma_start(out=iota_sb[:], in_=iotad[:, :])
            wt_sb = constp.tile([P, P], bf16)
            nc.sync.dma_start(out=wt_sb[:], in_=wt[:, :])
            norm_sb = constp.tile([P, n_tiles], f32)
            nc.sync.dma_start(out=norm_sb[:], in_=normt[:, :])

            res_g = None
            og = None
            idx_sb = None
            ic0 = 0
            for q, (t0, t1) in enumerate(quads):
                if q % G_IDX == 0:
                    g = q // G_IDX
                    ic0 = int(ig_col0[g])
                    ic1 = int(ig_col0[g + 1])
                    idx_sb = idxp.tile([P, max(ICGmax, 1)], i16, tag="idx")
                    nc.sync.dma_start(
                        out=idx_sb[:, : ic1 - ic0], in_=idxm[:, ic0:ic1]
                    )
                X = xp.tile([P, CQmax * P], bf16, tag="X")
                for b in range(nb):
                    L = int(L_qb[q, b])
                    if L == 0:
                        continue
                    cb = int(CB_qb[q, b])
                    g0 = int(gc0_qb[q, b])
                    io = int(icol_qb[q, b]) - ic0
                    icb = (L + 15) // 16
                    if L % P:
                        # the gather leaves partitions >= L%128 of its last
                        # chunk unwritten; pre-zero that chunk (on the ~idle
                        # ACT engine) so 0 * stale-NaN can't poison the
                        # one-hot matmul
                        nc.scalar.memzero(X[:, (g0 + cb - 1) * P : (g0 + cb) * P])
                    nc.gpsimd.dma_gather(
                        out_ap=X[:, g0 * P : (g0 + cb) * P].rearrange(
                            "p (c e) -> p c e", e=P
                        ),
                        in_ap=feats[b * B : min((b + 1) * B, n_nodes), :],
                        idxs_ap=idx_sb[:, io : io + icb],
                        num_idxs=L,
                        num_idxs_reg=L,
                        elem_size=P,
                        single_packet=False,
                        queue_num=b % 4,
                    )

                for t in range(t0, t1):
                    g = t // G_IO
                    j = t - g * G_IO
                    if j == 0:
                        gt = min(G_IO, n_tiles - g * G_IO)
                        full_t = (
                            gt if g < n_groups - 1 or rows_last == P else gt - 1
                        )
                        res_g = resp.tile([P, G_IO * P], f32, tag="res")
                        og = ogp.tile([P, G_IO * P], f32, tag="og")
                        r0 = g * G_IO * P
                        if full_t:
                            nc.sync.dma_start(
                                out=res_g[:, : full_t * P].rearrange(
                                    "p (t f) -> p t f", f=P
                                ),
                                in_=resid[r0 : r0 + full_t * P, :].rearrange(
                                    "(t p) f -> p t f", p=P
                                ),
                            )
                        if full_t < gt:
                            nc.sync.dma_start(
                                out=res_g[:rows_last, full_t * P : (full_t + 1) * P],
                                in_=resid[r0 + full_t * P : npc, :],
                            )

                    Wt = int(W_t[t])
                    L0 = int(L0_t[t])

                    # one-hot scatter matrix for the whole tile in ONE DVE
                    # op: S[e, (j,d)] = (ld[e,j] == d), ld broadcast along d
                    # via stride-0 AP
                    S_full = sp.tile([P, Wmax * P], bf16, tag="S")
                    S = S_full[:, : Wt * P]
                    ld_b = (
                        ld_sb[:, L0 : L0 + Wt]
                        .rearrange("p (c u) -> p c u", u=1)
                        .broadcast_to([P, Wt, P])
                    )
                    nc.vector.tensor_tensor(
                        out=S.rearrange("p (c e) -> p c e", e=P),
                        in0=ld_b,
                        in1=iota_sb[:, : Wt * P].rearrange("p (c e) -> p c e", e=P),
                        op=mybir.AluOpType.is_equal,
                    )

                    # zT[f, d] += X_c[e, f].T @ S_j[e, d]
                    z_ps = zpsp.tile([P, P], f32)
                    for wj, gc in enumerate(chunk_map[t]):
                        nc.tensor.matmul(
                            out=z_ps[:],
                            lhsT=X[:, gc * P : (gc + 1) * P],
                            rhs=S[:, wj * P : (wj + 1) * P],
                            start=(wj == 0),
                            stop=(wj == Wt - 1),
                        )

                    zT_sb = ztp.tile([P, P], bf16, tag="zT")
                    nc.vector.tensor_copy(out=zT_sb[:], in_=z_ps[:])
                    # y[d, o] = zT[f, d].T @ wt[f, o]
                    y_ps = ypsp.tile([P, P], f32)
                    nc.tensor.matmul(
                        out=y_ps[:], lhsT=zT_sb[:], rhs=wt_sb[:], start=True, stop=True
                    )

                    rows = P if t < n_tiles - 1 else rows_last
                    # fused ReLU(y * norm[dst]) with per-partition scale
                    y_sb = yp.tile([P, P], f32, tag="y")
                    nc.scalar.activation(
                        out=y_sb[:],
                        in_=y_ps[:],
                        func=mybir.ActivationFunctionType.Relu,
                        scale=norm_sb[:, t : t + 1],
                    )
                    nc.vector.tensor_add(
                        out=og[:rows, j * P : (j + 1) * P],
                        in0=y_sb[:rows],
                        in1=res_g[:rows, j * P : (j + 1) * P],
                    )

                    if j == G_IO - 1 or t == n_tiles - 1:
                        gt = j + 1
                        full_t = (
                            gt if t < n_tiles - 1 or rows_last == P else gt - 1
                        )
                        r0 = g * G_IO * P
                        if full_t:
                            nc.sync.dma_start(
                                out=out[r0 : r0 + full_t * P, :].rearrange(
                                    "(t p) f -> p t f", p=P
                                ),
                                in_=og[:, : full_t * P].rearrange(
                                    "p (t f) -> p t f", f=P
                                ),
                            )
                        if full_t < gt:
                            nc.sync.dma_start(
                                out=out[r0 + full_t * P : npc, :],
                                in_=og[:rows_last, full_t * P : (full_t + 1) * P],
                            )
    nc.finalize()
    return nc


def _run(features, W, edge_src, edge_dst, trace=False, **spmd_kwargs):
    in_maps, layout = _prepare(features, W, edge_src, edge_dst)
    nc = _build_program(layout)
    br = run_bass_kernel_spmd(
        nc, in_maps, core_ids=list(range(N_CORES)), trace=trace, **spmd_kwargs
    )
    outs = [r["out"] for r in br.results]
    full = np.concatenate(outs, axis=0).astype(np.float32)
    return full, br


def kernel(features, W, edge_src, edge_dst):
    out, _ = _run(features, W, edge_src, edge_dst, trace=False)
    return out
